# revision 15
# baseline (speedup 1.0000x reference)
"""Trainium2 Bass kernel for CausalSelfAttention (GQA + RMSNorm + partial RoPE).

Sharding: 8 cores = (batch b in 0..3) x (kv-head group g in 0..1).
Each core computes the full attention for its (b, g) slice and the partial
output projection over its head columns; the host sums the two partials per
batch and transposes back ([o, s] -> [s, o]).

All DRAM traffic and SBUF operands are bf16 (host pre-converts); PSUM
accumulation stays fp32.  Key structure per core:
  - QKV projections from xT [d, s] bf16 tiles; q in [s, o] layout for
    RMS-norm/RoPE, evacuated from PSUM with a single ACT copy, then
    PE-transposed to qT [hd, s]; k+v evacuated together into kv_sb with one
    Pool copy (k ops read the SBUF side; v slice feeds attn@v as lhsT).
  - k's rstd folds into the exp() per-partition scale; q's rstd*gain is
    multiplied in before RoPE (in place on the bf16 staging tile).
  - scoresT [sk, sq] = kT_tile.T @ qT per head; Exp on ACT (bf16 out, scale
    = rstd_k); causal via matmul range limits + one triangular mask multiply
    on the diagonal 128-block; denominator accumulated on DVE in bf16
    (2x 16-bit mode), reduced over partitions with a ones-matmul,
    reciprocal on DVE, partition-broadcast with a K=1 matmul, and applied
    by a DVE multiply reading both PSUM operands.
  - Output projection for block j inlined right after block j's softmax
    tail (borrows score-PSUM slots) so it overlaps the next block's
    ACT-heavy attention work; outT written bf16, host sums in fp32.
"""

import sys

for _p in ("/opt/trn_rl_repo",):
    if _p not in sys.path:
        sys.path.insert(0, _p)

import numpy as np
import ml_dtypes

import concourse.bass as bass
import concourse.bacc as bacc
import concourse.mybir as mybir
import concourse.tile as tile
from concourse import bass_utils
from concourse.masks import make_identity

F32 = mybir.dt.float32
BF16 = mybir.dt.bfloat16
BFNP = np.dtype(ml_dtypes.bfloat16)
AFT = mybir.ActivationFunctionType

B, S, D = 4, 2048, 1024
H, KVH, HD = 8, 2, 128
NH = H // KVH          # q heads per core = 4
RD, RH = 64, 32        # rope dims / half
NB, BLK = 4, 512       # s blocks
NT, TS = 16, 128       # s tiles
NDC = D // 128         # 8 d-chunks
EPS = float(np.finfo(np.float32).eps)


def _build_nc(reps=1):
    nc = bacc.Bacc("TRN2", target_bir_lowering=False, debug=False,
                   enable_asserts=False)

    xT = nc.dram_tensor("xT", (D, S), BF16, kind="ExternalInput").ap()
    wq = nc.dram_tensor("wq", (128, NDC, NH * HD), BF16,
                        kind="ExternalInput").ap()
    wkv = nc.dram_tensor("wkv", (128, NDC, 2 * HD), BF16,
                         kind="ExternalInput").ap()
    wo = nc.dram_tensor("wo", (128, NH, D), BF16, kind="ExternalInput").ap()
    cosq = nc.dram_tensor("cosq", (128, NT, RH), BF16,
                          kind="ExternalInput").ap()
    sinq = nc.dram_tensor("sinq", (128, NT, RH), BF16,
                          kind="ExternalInput").ap()
    nsinq = nc.dram_tensor("nsinq", (128, NT, RH), BF16,
                           kind="ExternalInput").ap()
    qsc = nc.dram_tensor("qsc", (1, NH), F32, kind="ExternalInput").ap()
    outT = nc.dram_tensor("outT", (D, S), BF16, kind="ExternalOutput").ap()

    with tile.TileContext(nc) as tc, \
         nc.allow_low_precision(reason="bf16 attention"):
        for _rep in range(reps):
            _kern(nc, tc, xT, wq, wkv, wo, cosq, sinq, nsinq, qsc, outT)
    nc.compile()
    return nc


def _kern(nc, tc, xT, wq, wkv, wo, cosq, sinq, nsinq, qsc, outT):
    mm = nc.tensor.matmul

    persist_cm = tc.tile_pool(name="persist", bufs=1)
    persist = persist_cm.__enter__()
    # ---- persistent tiles -------------------------------------------------
    wq_sb = persist.tile([128, NDC, NH * HD], BF16, tag="wq_sb", name="wq_sb")
    wkv_sb = persist.tile([128, NDC, 2 * HD], BF16, tag="wkv_sb",
                          name="wkv_sb")
    # Weights go in per-chunk DMAs (so the first QKV matmuls can start
    # before the whole tensors land), split across the two HW DGE queues
    # (SP carries wkv, ACT carries wq + tables) to halve descriptor-gen
    # latency at startup.  The x tiles for b=0 are interleaved on SP below.
    wo_sb = persist.tile([128, NH, D], BF16, tag="wo_sb", name="wo_sb")
    cosq_sb = persist.tile([128, NT, RH], BF16, tag="cosq_sb", name="cosq_sb")
    sinq_sb = persist.tile([128, NT, RH], BF16, tag="sinq_sb", name="sinq_sb")
    nsinq_sb = persist.tile([128, NT, RH], BF16, tag="nsinq_sb",
                            name="nsinq_sb")
    qsc_sb = persist.tile([128, NH], F32, tag="qsc_sb", name="qsc_sb")
    for di in range(NDC):
        nc.scalar.dma_start(out=wq_sb[:, di, :], in_=wq[:, di, :])
    nc.scalar.dma_start(out=cosq_sb, in_=cosq)
    nc.scalar.dma_start(out=sinq_sb, in_=sinq)
    nc.scalar.dma_start(out=nsinq_sb, in_=nsinq)
    nc.scalar.dma_start(out=qsc_sb, in_=qsc.to_broadcast((128, NH)))

    ones_col = persist.tile([128, 1], BF16, tag="ones_col", name="ones_col")
    nc.vector.memset(ones_col, 1.0)
    ones_row = persist.tile([1, 128], BF16, tag="ones_row", name="ones_row")
    nc.vector.memset(ones_row, 1.0)
    eps_col = persist.tile([128, 1], F32, tag="eps_col", name="eps_col")
    nc.vector.memset(eps_col, EPS)
    ident_st = persist.tile([128, 128], F32, tag="ident_st", name="ident_st")
    make_identity(nc, ident_st)
    ident = persist.tile([128, 128], BF16, tag="ident", name="ident")
    nc.vector.tensor_copy(out=ident, in_=ident_st)
    # tri[r, c] = 1.0 if r <= c else 0.0  (causal keep-mask on the diagonal)
    tri_st = persist.tile([128, 128], F32, tag="tri_st", name="tri_st")
    nc.gpsimd.memset(tri_st, 1.0)
    nc.gpsimd.affine_select(
        out=tri_st, in_=tri_st, compare_op=mybir.AluOpType.is_ge, fill=0.0,
        base=0, pattern=[[1, 128]], channel_multiplier=-1)
    tri = persist.tile([128, 128], BF16, tag="tri", name="tri")
    nc.vector.tensor_copy(out=tri, in_=tri_st)

    qT_sb = persist.tile([128, NH, S], BF16, tag="qT_sb", name="qT_sb")
    kT_sb = persist.tile([128, S], BF16, tag="kT_sb", name="kT_sb")
    kv_sb = persist.tile([128, NT, 2 * HD], BF16, tag="kv_sb", name="kv_sb")
    rstdk_sb = persist.tile([128, NT], F32, tag="rstdk_sb", name="rstdk_sb")
    yT_sb = persist.tile([128, NH, S], BF16, tag="yT_sb", name="yT_sb")

    # ---- phase 1: projections + norm + rope + transposes ------------------
    # PSUM budget (8 banks): q_ps 4 + kv_ps 2 + misc_ps 2.
    with tc.tile_pool(name="p1_psum", bufs=1, space="PSUM") as p1ps, \
         tc.tile_pool(name="p1_sbuf", bufs=1) as p1sb:

        def proc_q(i, qp):
            # qp: PSUM [128, 512] f32 = q rows for s-tile i, 4 heads x hd.
            # Single ACT copy evacuates PSUM; everything else reads bf16 SBUF.
            qraw = p1sb.tile([128, BLK], BF16, tag="qraw", bufs=3,
                             name=f"qraw_{i}")
            nc.scalar.activation(out=qraw, in_=qp, func=AFT.Copy)
            qrv = qraw.rearrange("p (h f) -> p h f", h=NH)
            sq = p1sb.tile([128, BLK], BF16, tag="sq", bufs=3,
                           name=f"sq_{i}")
            nc.scalar.activation(out=sq, in_=qraw, func=AFT.Square)
            sumsq = p1sb.tile([128, NH], F32, tag="sumsq", bufs=3,
                              name=f"sumsq_{i}")
            nc.vector.tensor_reduce(
                out=sumsq, in_=sq.rearrange("p (h f) -> p h f", h=NH),
                axis=mybir.AxisListType.X, op=mybir.AluOpType.add)
            qsrt = p1sb.tile([128, NH], F32, tag="qsrt", bufs=3,
                             name=f"qsrt_{i}")
            nc.scalar.activation(out=qsrt, in_=sumsq, func=AFT.Sqrt,
                                 bias=eps_col, scale=1.0 / HD)
            rstd = p1sb.tile([128, NH], F32, tag="rstd", bufs=3,
                             name=f"rstd_{i}")
            nc.vector.reciprocal(rstd, qsrt)
            rsc = p1sb.tile([128, NH], F32, tag="rsc", bufs=3,
                            name=f"rsc_{i}")
            nc.vector.tensor_mul(rsc, rstd, qsc_sb)

            # qn = qraw * (rstd*gain*HD^-0.5); RoPE applied in place on qn
            qn = p1sb.tile([128, BLK], BF16, tag="qn", bufs=3,
                           name=f"qn_{i}")
            qnv = qn.rearrange("p (h f) -> p h f", h=NH)
            nc.gpsimd.tensor_mul(
                qnv, qrv, rsc[:, :, None].broadcast_to([128, NH, HD]))
            cos4 = cosq_sb[:, i:i + 1, None, :].broadcast_to(
                [128, NH, 2, RH])
            sin3 = sinq_sb[:, i:i + 1, :].broadcast_to([128, NH, RH])
            nsin3 = nsinq_sb[:, i:i + 1, :].broadcast_to([128, NH, RH])
            tcq = p1sb.tile([128, NH, RD], BF16, tag="tcq", bufs=3,
                            name=f"tcq_{i}")
            nc.vector.tensor_mul(
                tcq.rearrange("p h (two f) -> p h two f", two=2),
                qnv[:, :, 0:RD].rearrange("p h (two f) -> p h two f", two=2),
                cos4)
            tsq = p1sb.tile([128, NH, RD], BF16, tag="tsq", bufs=3,
                            name=f"tsq_{i}")
            nc.vector.tensor_mul(tsq[:, :, 0:RH], qnv[:, :, RH:RD], sin3)
            nc.vector.tensor_mul(tsq[:, :, RH:RD], qnv[:, :, 0:RH], nsin3)
            nc.vector.tensor_add(qnv[:, :, 0:RD], tcq, tsq)
            for h in range(NH):
                qt_ps = p1ps.tile([128, 128], BF16, tag="misc_ps", bufs=2,
                                  name=f"qtps_{i}_{h}")
                nc.tensor.transpose(qt_ps, qn[:, h * 128:(h + 1) * 128],
                                    ident)
                nc.vector.tensor_copy(
                    out=qT_sb[:, h, i * 128:(i + 1) * 128], in_=qt_ps)

        def proc_k(t):
            # reads k half of kv_sb[:, t, :] (bf16 SBUF)
            kn = kv_sb[:, t, 0:HD]
            ksq = p1sb.tile([128, HD], BF16, tag="ksq", bufs=3,
                            name=f"ksq_{t}")
            ksum = p1sb.tile([128, 1], F32, tag="ksum", bufs=3,
                             name=f"ksum_{t}")
            nc.scalar.activation(out=ksq, in_=kn, func=AFT.Square,
                                 accum_out=ksum)
            ksrt = p1sb.tile([128, 1], F32, tag="ksrt", bufs=3,
                             name=f"ksrt_{t}")
            nc.scalar.activation(out=ksrt, in_=ksum, func=AFT.Sqrt,
                                 bias=eps_col, scale=1.0 / HD)
            nc.vector.reciprocal(rstdk_sb[:, t:t + 1], ksrt)
            cos2 = cosq_sb[:, t:t + 1, :].broadcast_to([128, 2, RH])
            tck = p1sb.tile([128, RD], BF16, tag="tck", bufs=3,
                            name=f"tck_{t}")
            nc.vector.tensor_mul(
                tck.rearrange("p (two f) -> p two f", two=2),
                kn[:, 0:RD].rearrange("p (two f) -> p two f", two=2), cos2)
            tsk = p1sb.tile([128, RD], BF16, tag="tsk", bufs=3,
                            name=f"tsk_{t}")
            nc.vector.tensor_mul(tsk[:, 0:RH], kn[:, RH:RD],
                                 sinq_sb[:, t, :])
            nc.vector.tensor_mul(tsk[:, RH:RD], kn[:, 0:RH],
                                 nsinq_sb[:, t, :])
            kst = p1sb.tile([128, HD], BF16, tag="kst", bufs=3,
                            name=f"kst_{t}")
            nc.vector.tensor_add(kst[:, 0:RD], tck, tsk)
            nc.vector.tensor_copy(out=kst[:, RD:HD], in_=kn[:, RD:HD])
            kt_ps = p1ps.tile([128, 128], BF16, tag="misc_ps", bufs=2,
                              name=f"ktps_{t}")
            nc.tensor.transpose(kt_ps, kst, ident)
            nc.scalar.activation(out=kT_sb[:, t * 128:(t + 1) * 128],
                                 in_=kt_ps, func=AFT.Copy)

        for b in range(NB):
            sl = slice(b * BLK, (b + 1) * BLK)
            xts = []
            for di in range(NDC):
                xt = p1sb.tile([128, BLK], BF16, tag="xt", bufs=16,
                               name=f"xt_{b}_{di}")
                nc.sync.dma_start(out=xt,
                                  in_=xT[di * 128:(di + 1) * 128, sl])
                if b == 0:
                    nc.sync.dma_start(out=wkv_sb[:, di, :], in_=wkv[:, di, :])
                xts.append(xt)
            if b == 0:
                # wo isn't needed until the first out-projection; issuing its
                # DMA after the first x block keeps startup DMA for phase 1.
                nc.sync.dma_start(out=wo_sb, in_=wo)
            for half in range(2):
                qps2 = [p1ps.tile([128, BLK], F32, tag="q_ps", bufs=4,
                                  name=f"qps_{b}_{half}_{x}")
                        for x in range(2)]
                kvps2 = [p1ps.tile([128, 2 * HD], F32, tag="kv_ps", bufs=2,
                                   name=f"kvps_{b}_{half}_{x}")
                         for x in range(2)]
                for di in range(NDC):
                    st, sp = di == 0, di == NDC - 1
                    for x in range(2):
                        ii = half * 2 + x
                        xsl = xts[di][:, ii * 128:(ii + 1) * 128]
                        mm(qps2[x], lhsT=xsl, rhs=wq_sb[:, di, :],
                           start=st, stop=sp)
                        mm(kvps2[x], lhsT=xsl, rhs=wkv_sb[:, di, :],
                           start=st, stop=sp)
                for x in range(2):
                    i = b * 4 + half * 2 + x
                    # single DVE copy frees the kv PSUM tile quickly
                    nc.vector.tensor_copy(out=kv_sb[:, i, :], in_=kvps2[x])
                    proc_k(i)
                    proc_q(i, qps2[x])

    # ---- phase 2+3: attention + per-block output projection ---------------
    # PSUM (8 banks): sc_ps [128,2,512] x2 (4 banks) + y_ps [128,2,512] x2
    # (4 banks); tail dn/rdb and out-projection tiles borrow sc_ps slots.
    with tc.tile_pool(name="p2_psum", bufs=1, space="PSUM") as p2ps, \
         tc.tile_pool(name="p2_sbuf", bufs=1) as p2sb:

        def outproj(j, oc):
            def thunk():
                jsl = slice(j * BLK, (j + 1) * BLK)
                osl = slice(oc * 128, (oc + 1) * 128)
                ops_ = p2ps.tile([128, BLK], F32, tag="aux_ps", bufs=2,
                                 name=f"ops_{j}_{oc}")
                for c in range(NH):
                    mm(ops_, lhsT=wo_sb[:, c, osl], rhs=yT_sb[:, c, jsl],
                       start=(c == 0), stop=(c == NH - 1))
                oc_sb = p2sb.tile([128, BLK], BF16, tag="oc_sb", bufs=6,
                                  name=f"ocsb_{j}_{oc}")
                if oc % 2 == 0:
                    nc.scalar.activation(out=oc_sb, in_=ops_, func=AFT.Copy)
                else:
                    nc.vector.tensor_copy(out=oc_sb, in_=ops_)
                nc.sync.dma_start(out=outT[osl, jsl], in_=oc_sb)
            return thunk

        pending = []
        for j in range(NB):
            n_t = 4 * j + 4
            jsl = slice(j * BLK, (j + 1) * BLK)
            work, wi = pending, 0
            for hp in range(2):          # head-pair pass (heads 2hp, 2hp+1)
                yps = p2ps.tile([128, 2, BLK], F32, tag="y_ps", bufs=1,
                                name=f"yps_{j}_{hp}")
                den_e = p2sb.tile([128, 2, BLK], BF16, tag="den_e", bufs=2,
                                  name=f"dene_{j}_{hp}")
                den_o = p2sb.tile([128, 2, BLK], BF16, tag="den_o", bufs=2,
                                  name=f"deno_{j}_{hp}")
                for t in range(n_t):
                    diag = t >= 4 * j
                    m = (t - 4 * j) * 128 if diag else 0
                    tsl = slice(t * 128, (t + 1) * 128)
                    sc2 = p2ps.tile([128, 2, BLK], F32, tag="sc_ps",
                                    bufs=2, name=f"sc2_{j}_{hp}_{t}")
                    et2 = p2sb.tile([128, 2, BLK], BF16, tag="expt",
                                    bufs=6, name=f"expt_{j}_{hp}_{t}")
                    for hh in range(2):
                        h = hp * 2 + hh
                        mm(sc2[:, hh, m:BLK], lhsT=kT_sb[:, tsl],
                           rhs=qT_sb[:, h, j * BLK + m:(j + 1) * BLK],
                           start=True, stop=True)
                    nc.scalar.activation(out=et2[:, :, m:BLK],
                                         in_=sc2[:, :, m:BLK], func=AFT.Exp,
                                         scale=rstdk_sb[:, t:t + 1])
                    if diag:
                        nc.gpsimd.tensor_mul(
                            et2[:, :, m:m + 128], et2[:, :, m:m + 128],
                            tri[:, None, :].broadcast_to([128, 2, 128]))
                    if t == 0:
                        nc.vector.tensor_copy(out=den_e, in_=et2)
                    elif j == 0 or t % 2 == 0:
                        nc.vector.tensor_add(den_e[:, :, m:BLK],
                                             den_e[:, :, m:BLK],
                                             et2[:, :, m:BLK])
                    elif t == 1:
                        nc.vector.tensor_copy(out=den_o, in_=et2)
                    else:
                        nc.vector.tensor_add(den_o[:, :, m:BLK],
                                             den_o[:, :, m:BLK],
                                             et2[:, :, m:BLK])
                    for hh in range(2):
                        mm(yps[:, hh, m:BLK], lhsT=kv_sb[:, t, HD:2 * HD],
                           rhs=et2[:, hh, m:BLK],
                           start=(t == 0), stop=(t == n_t - 1))
                    if wi < len(work):
                        work[wi]()
                        wi += 1
                # ---- softmax tail for this pass's two heads ----
                for hh in range(2):
                    h = hp * 2 + hh
                    dn_ps = p2ps.tile([1, BLK], F32, tag="aux_ps", bufs=2,
                                      name=f"dnps_{j}_{h}")
                    mm(dn_ps, lhsT=ones_col, rhs=den_e[:, hh, :],
                       start=True, stop=(j == 0))
                    if j > 0:
                        mm(dn_ps, lhsT=ones_col, rhs=den_o[:, hh, :],
                           start=False, stop=True)
                    rdr = p2sb.tile([1, BLK], BF16, tag="rdr", bufs=4,
                                    name=f"rdr_{j}_{h}")
                    nc.vector.reciprocal(rdr, dn_ps)
                    rdb_ps = p2ps.tile([128, BLK], F32, tag="aux_ps",
                                       bufs=2, name=f"rdbps_{j}_{h}")
                    mm(rdb_ps, lhsT=ones_row, rhs=rdr, start=True,
                       stop=True)
                    # HW allows only one PSUM input per vector op: stage the
                    # broadcast reciprocal in SBUF before the normalize mul
                    rdb_sb = p2sb.tile([128, BLK], BF16, tag="rdb_sb",
                                       bufs=4, name=f"rdbsb_{j}_{h}")
                    nc.scalar.activation(out=rdb_sb, in_=rdb_ps,
                                         func=AFT.Copy)
                    nc.vector.tensor_mul(yT_sb[:, h, jsl], yps[:, hh, :],
                                         rdb_sb)
            while wi < len(work):
                work[wi]()
                wi += 1
            pending = [outproj(j, oc) for oc in range(8)]
        for thunk in pending:
            thunk()

    persist_cm.__exit__(None, None, None)


_NC_CACHE = {}


def _get_nc():
    if "nc" not in _NC_CACHE:
        _NC_CACHE["nc"] = _build_nc()
    return _NC_CACHE["nc"]


def _host_tables():
    pos = np.arange(S, dtype=np.float32)
    inv = (1.0 / (10000.0 ** (np.arange(0, RD, 2, dtype=np.float32) / RD)))
    fr = np.outer(pos, inv).astype(np.float32)          # [S, 32]
    cos, sin = np.cos(fr), np.sin(fr)
    tile128 = lambda a: np.ascontiguousarray(
        a.reshape(NT, 128, RH).transpose(1, 0, 2)).astype(BFNP)
    return tile128(cos), tile128(sin), tile128(-sin)


def core_in_map(x, w_q, w_k, w_v, w_o, q_gain, core):
    """Host-side per-core input prep: shard + transpose + bf16 convert."""
    cosq, sinq, nsinq = _host_tables()
    b, g = divmod(core, 2)
    cols = slice(g * NH * HD, (g + 1) * NH * HD)

    def wtile(wT, chunks, width):
        # [chunks*128, width] -> [128, chunks, width] bf16
        return np.ascontiguousarray(
            wT.reshape(chunks, 128, width).transpose(1, 0, 2)).astype(BFNP)

    xTc = np.ascontiguousarray(x[b].T).astype(BFNP)             # [D, S]
    wq_t = wtile(np.ascontiguousarray(w_q[cols, :].T), NDC, NH * HD)
    wkv_t = wtile(np.ascontiguousarray(np.concatenate(
        [w_k[g * HD:(g + 1) * HD, :].T, w_v[g * HD:(g + 1) * HD, :].T],
        axis=1)), NDC, 2 * HD)
    wo_t = wtile(np.ascontiguousarray(w_o[:, cols].T), NH, D)
    qsc_h = (q_gain[g * NH:(g + 1) * NH] *
             np.float32(HD ** -0.5)).astype(np.float32).reshape(1, NH)
    return dict(xT=xTc, wq=wq_t, wkv=wkv_t, wo=wo_t,
                cosq=cosq, sinq=sinq, nsinq=nsinq, qsc=qsc_h)


def kernel(x, w_q, w_k, w_v, w_o, q_gain):
    x = np.asarray(x, dtype=np.float32)
    w_q = np.asarray(w_q, dtype=np.float32)
    w_k = np.asarray(w_k, dtype=np.float32)
    w_v = np.asarray(w_v, dtype=np.float32)
    w_o = np.asarray(w_o, dtype=np.float32)
    q_gain = np.asarray(q_gain, dtype=np.float32)

    nc = _get_nc()
    in_maps = [core_in_map(x, w_q, w_k, w_v, w_o, q_gain, core)
               for core in range(8)]
    res = bass_utils.run_bass_kernel_spmd(nc, in_maps,
                                          core_ids=list(range(8)))
    out = np.empty((B, S, D), dtype=np.float32)
    for b in range(B):
        p0 = res.results[2 * b]["outT"].astype(np.float32)
        p1 = res.results[2 * b + 1]["outT"].astype(np.float32)
        out[b] = (p0 + p1).T
    return out


# revision 34
# speedup vs baseline: 1.3241x; 1.3241x over previous
"""Trainium2 Bass kernel for CausalSelfAttention (GQA + RMSNorm + partial RoPE).

Sharding: 8 cores = (batch b in 0..3) x (kv-head group g in 0..1).
Each core computes the full attention for its (b, g) slice and the partial
output projection over its head columns; the host sums the two partials per
batch and transposes back ([o, s] -> [s, o]).

All DRAM traffic and SBUF operands are bf16 (host pre-converts); PSUM
accumulation stays fp32.  Key structure per core:
  - QKV projections from xT [d, s] bf16 tiles; q in [s, o] layout for
    RMS-norm/RoPE, evacuated from PSUM with a single ACT copy, then
    PE-transposed to qT [hd, s]; k+v evacuated together into kv_sb with one
    Pool copy (k ops read the SBUF side; v slice feeds attn@v as lhsT).
  - k's rstd folds into the exp() per-partition scale; q's rstd*gain is
    multiplied in before RoPE (in place on the bf16 staging tile).
  - scoresT [sk, sq] = kT_tile.T @ qT per head; Exp on ACT (bf16 out, scale
    = rstd_k); causal via matmul range limits + one triangular mask multiply
    on the diagonal 128-block; denominator accumulated on DVE in bf16
    (2x 16-bit mode), reduced over partitions with a ones-matmul,
    reciprocal on DVE, partition-broadcast with a K=1 matmul, and applied
    by a DVE multiply reading both PSUM operands.
  - Output projection for block j inlined right after block j's softmax
    tail (borrows score-PSUM slots) so it overlaps the next block's
    ACT-heavy attention work; outT written bf16, host sums in fp32.
"""

import sys

for _p in ("/opt/trn_rl_repo",):
    if _p not in sys.path:
        sys.path.insert(0, _p)

import numpy as np
import ml_dtypes

import concourse.bass as bass
import concourse.bacc as bacc
import concourse.mybir as mybir
import concourse.tile as tile
from concourse import bass_utils
from concourse.masks import make_identity

F32 = mybir.dt.float32
BF16 = mybir.dt.bfloat16
BFNP = np.dtype(ml_dtypes.bfloat16)
AFT = mybir.ActivationFunctionType

# Pin the ACT table to natural_log_exp_and_others (exp+ln+copy+square — the
# only funcs this kernel uses) so the table chooser can't ping-pong between
# the exp-only and ln+exp sets, which costs a 1.3us table load each flip.
# Positional ids must be preserved, so other sets are blanked, not removed.
_orig_get_act_tables = bacc.get_activation_tables


def _pinned_act_tables(arch):
    tabs = _orig_get_act_tables(arch)
    return {name: (funcs if name == "natural_log_exp_and_others" else set())
            for name, funcs in tabs.items()}


bacc.get_activation_tables = _pinned_act_tables

B, S, D = 4, 2048, 1024
H, KVH, HD = 8, 2, 128
NH = H // KVH          # q heads per core = 4
RD, RH = 64, 32        # rope dims / half
NB, BLK = 4, 512       # s blocks
NT, TS = 16, 128       # s tiles
NDC = D // 128         # 8 d-chunks
EPS = float(np.finfo(np.float32).eps)


def _build_nc(reps=1):
    nc = bacc.Bacc("TRN2", target_bir_lowering=False, debug=False,
                   enable_asserts=False)

    xT = nc.dram_tensor("xT", (D, S), BF16, kind="ExternalInput").ap()
    wq = nc.dram_tensor("wq", (128, NDC, NH * HD), BF16,
                        kind="ExternalInput").ap()
    wkv = nc.dram_tensor("wkv", (128, NDC, 2 * HD), BF16,
                         kind="ExternalInput").ap()
    wo = nc.dram_tensor("wo", (128, NH, D), BF16, kind="ExternalInput").ap()
    cosq = nc.dram_tensor("cosq", (128, NT, RH), BF16,
                          kind="ExternalInput").ap()
    sinq = nc.dram_tensor("sinq", (128, NT, RH), BF16,
                          kind="ExternalInput").ap()
    nsinq = nc.dram_tensor("nsinq", (128, NT, RH), BF16,
                           kind="ExternalInput").ap()
    qsc = nc.dram_tensor("qsc", (1, NH), F32, kind="ExternalInput").ap()
    outT = nc.dram_tensor("outT", (D, S), BF16, kind="ExternalOutput").ap()

    with tile.TileContext(nc) as tc, \
         nc.allow_low_precision(reason="bf16 attention"):
        for _rep in range(reps):
            _kern(nc, tc, xT, wq, wkv, wo, cosq, sinq, nsinq, qsc, outT)
    nc.compile()
    return nc


def _kern(nc, tc, xT, wq, wkv, wo, cosq, sinq, nsinq, qsc, outT):
    mm = nc.tensor.matmul

    persist_cm = tc.tile_pool(name="persist", bufs=1)
    persist = persist_cm.__enter__()
    # ---- persistent tiles -------------------------------------------------
    wq_sb = persist.tile([128, NDC, NH * HD], BF16, tag="wq_sb", name="wq_sb")
    wkv_sb = persist.tile([128, NDC, 2 * HD], BF16, tag="wkv_sb",
                          name="wkv_sb")
    # Weights go in per-chunk DMAs (so the first QKV matmuls can start
    # before the whole tensors land), split across the two HW DGE queues
    # (SP carries wkv, ACT carries wq + tables) to halve descriptor-gen
    # latency at startup.  The x tiles for b=0 are interleaved on SP below.
    wo_sb = persist.tile([128, NH, D], BF16, tag="wo_sb", name="wo_sb")
    cosq_sb = persist.tile([128, NT, RH], BF16, tag="cosq_sb", name="cosq_sb")
    sinq_sb = persist.tile([128, NT, RH], BF16, tag="sinq_sb", name="sinq_sb")
    nsinq_sb = persist.tile([128, NT, RH], BF16, tag="nsinq_sb",
                            name="nsinq_sb")
    qsc_sb = persist.tile([128, NH], F32, tag="qsc_sb", name="qsc_sb")
    for di in range(NDC):
        nc.scalar.dma_start(out=wq_sb[:, di, :], in_=wq[:, di, :])
    nc.scalar.dma_start(out=cosq_sb, in_=cosq)
    nc.scalar.dma_start(out=sinq_sb, in_=sinq)
    nc.scalar.dma_start(out=nsinq_sb, in_=nsinq)
    nc.scalar.dma_start(out=qsc_sb, in_=qsc.to_broadcast((128, NH)))

    ones_col = persist.tile([128, 1], BF16, tag="ones_col", name="ones_col")
    nc.vector.memset(ones_col, 1.0)
    ones_row = persist.tile([1, 128], BF16, tag="ones_row", name="ones_row")
    nc.vector.memset(ones_row, 1.0)
    eps_col = persist.tile([128, 1], F32, tag="eps_col", name="eps_col")
    nc.vector.memset(eps_col, EPS)
    ident_st = persist.tile([128, 128], F32, tag="ident_st", name="ident_st")
    make_identity(nc, ident_st)
    ident = persist.tile([128, 128], BF16, tag="ident", name="ident")
    nc.vector.tensor_copy(out=ident, in_=ident_st)
    # tri[r, c] = 1.0 if r <= c else 0.0  (causal keep-mask on the diagonal)
    tri_st = persist.tile([128, 128], F32, tag="tri_st", name="tri_st")
    nc.gpsimd.memset(tri_st, 1.0)
    nc.gpsimd.affine_select(
        out=tri_st, in_=tri_st, compare_op=mybir.AluOpType.is_ge, fill=0.0,
        base=0, pattern=[[1, 128]], channel_multiplier=-1)
    tri = persist.tile([128, 128], BF16, tag="tri", name="tri")
    nc.vector.tensor_copy(out=tri, in_=tri_st)

    qT_sb = persist.tile([128, NH, S], BF16, tag="qT_sb", name="qT_sb")
    kT_sb = persist.tile([128, S], BF16, tag="kT_sb", name="kT_sb")
    kv_sb = persist.tile([128, NT, 2 * HD], BF16, tag="kv_sb", name="kv_sb")
    rstdk_sb = persist.tile([128, NT], F32, tag="rstdk_sb", name="rstdk_sb")
    yT_sb = persist.tile([128, NH, S], BF16, tag="yT_sb", name="yT_sb")

    # ---- phase 1: projections + norm + rope + transposes ------------------
    # PSUM budget (8 banks): q_ps 4 + kv_ps 2 + misc_ps 2.
    with tc.tile_pool(name="p1_psum", bufs=1, space="PSUM") as p1ps, \
         tc.tile_pool(name="p1_sbuf", bufs=1) as p1sb:

        def proc_q(i, qp):
            # qp: PSUM [128, 512] f32 = q rows for s-tile i, 4 heads x hd.
            # Single ACT copy evacuates PSUM; everything else reads bf16 SBUF.
            qraw = p1sb.tile([128, BLK], BF16, tag="qraw", bufs=3,
                             name=f"qraw_{i}")
            nc.scalar.activation(out=qraw, in_=qp, func=AFT.Copy)
            qrv = qraw.rearrange("p (h f) -> p h f", h=NH)
            sq = p1sb.tile([128, BLK], BF16, tag="sq", bufs=3,
                           name=f"sq_{i}")
            nc.scalar.activation(out=sq, in_=qraw, func=AFT.Square)
            sumsq = p1sb.tile([128, NH], F32, tag="sumsq", bufs=3,
                              name=f"sumsq_{i}")
            nc.vector.tensor_reduce(
                out=sumsq, in_=sq.rearrange("p (h f) -> p h f", h=NH),
                axis=mybir.AxisListType.X, op=mybir.AluOpType.add)
            # rstd = exp(-0.5*ln(mean+eps)): stays in the exp/ln ACT table
            # set, so the whole kernel never reloads activation tables
            ql = p1sb.tile([128, NH], F32, tag="ql", bufs=3,
                           name=f"ql_{i}")
            nc.scalar.activation(out=ql, in_=sumsq, func=AFT.Ln,
                                 bias=eps_col, scale=1.0 / HD)
            rstd = p1sb.tile([128, NH], F32, tag="rstd", bufs=3,
                             name=f"rstd_{i}")
            nc.scalar.activation(out=rstd, in_=ql, func=AFT.Exp, scale=-0.5)
            rsc = p1sb.tile([128, NH], F32, tag="rsc", bufs=3,
                            name=f"rsc_{i}")
            nc.vector.tensor_mul(rsc, rstd, qsc_sb)

            # RoPE on the raw q (independent of the rstd chain); the
            # rstd*gain scale lands in the final two Pool multiplies only
            cos4 = cosq_sb[:, i:i + 1, None, :].broadcast_to(
                [128, NH, 2, RH])
            sin3 = sinq_sb[:, i:i + 1, :].broadcast_to([128, NH, RH])
            nsin3 = nsinq_sb[:, i:i + 1, :].broadcast_to([128, NH, RH])
            tcq = p1sb.tile([128, NH, RD], BF16, tag="tcq", bufs=3,
                            name=f"tcq_{i}")
            nc.vector.tensor_mul(
                tcq.rearrange("p h (two f) -> p h two f", two=2),
                qrv[:, :, 0:RD].rearrange("p h (two f) -> p h two f", two=2),
                cos4)
            tsq = p1sb.tile([128, NH, RD], BF16, tag="tsq", bufs=3,
                            name=f"tsq_{i}")
            nc.vector.tensor_mul(tsq[:, :, 0:RH], qrv[:, :, RH:RD], sin3)
            nc.vector.tensor_mul(tsq[:, :, RH:RD], qrv[:, :, 0:RH], nsin3)
            nc.vector.tensor_add(tcq, tcq, tsq)
            qn = p1sb.tile([128, BLK], BF16, tag="qn", bufs=3,
                           name=f"qn_{i}")
            qnv = qn.rearrange("p (h f) -> p h f", h=NH)
            rsc_b = rsc[:, :, None].broadcast_to([128, NH, RD])
            nc.gpsimd.tensor_mul(qnv[:, :, 0:RD], tcq, rsc_b)
            nc.gpsimd.tensor_mul(qnv[:, :, RD:HD], qrv[:, :, RD:HD], rsc_b)
            qt_ps = p1ps.tile([128, NH, 128], BF16, tag="kv_ps", bufs=1,
                              name=f"qtps_{i}")
            for h in range(NH):
                nc.tensor.transpose(qt_ps[:, h, :],
                                    qn[:, h * 128:(h + 1) * 128], ident)
            nc.vector.tensor_copy(
                out=qT_sb[:, :, i * 128:(i + 1) * 128], in_=qt_ps)

        def proc_k(t, late=False):
            v_eng = nc.gpsimd if late else nc.vector
            # reads k half of kv_sb[:, t, :] (bf16 SBUF)
            kn = kv_sb[:, t, 0:HD]
            ksq = p1sb.tile([128, HD], BF16, tag="ksq", bufs=3,
                            name=f"ksq_{t}")
            ksum = p1sb.tile([128, 1], F32, tag="ksum", bufs=3,
                             name=f"ksum_{t}")
            nc.scalar.activation(out=ksq, in_=kn, func=AFT.Square,
                                 accum_out=ksum)
            kl = p1sb.tile([128, 1], F32, tag="kl", bufs=3,
                           name=f"kl_{t}")
            nc.scalar.activation(out=kl, in_=ksum, func=AFT.Ln,
                                 bias=eps_col, scale=1.0 / HD)
            nc.scalar.activation(out=rstdk_sb[:, t:t + 1], in_=kl,
                                 func=AFT.Exp, scale=-0.5)
            cos2 = cosq_sb[:, t:t + 1, :].broadcast_to([128, 2, RH])
            tck = p1sb.tile([128, RD], BF16, tag="tck", bufs=3,
                            name=f"tck_{t}")
            v_eng.tensor_mul(
                tck.rearrange("p (two f) -> p two f", two=2),
                kn[:, 0:RD].rearrange("p (two f) -> p two f", two=2), cos2)
            tsk = p1sb.tile([128, RD], BF16, tag="tsk", bufs=3,
                            name=f"tsk_{t}")
            v_eng.tensor_mul(tsk[:, 0:RH], kn[:, RH:RD],
                                 sinq_sb[:, t, :])
            v_eng.tensor_mul(tsk[:, RH:RD], kn[:, 0:RH],
                                 nsinq_sb[:, t, :])
            kst = p1sb.tile([128, HD], BF16, tag="kst", bufs=3,
                            name=f"kst_{t}")
            v_eng.tensor_add(kst[:, 0:RD], tck, tsk)
            v_eng.tensor_copy(out=kst[:, RD:HD], in_=kn[:, RD:HD])
            kt_ps = p1ps.tile([128, 128], BF16, tag="kv_ps", bufs=1,
                              name=f"ktps_{t}")
            nc.tensor.transpose(kt_ps, kst, ident)
            nc.scalar.activation(out=kT_sb[:, t * 128:(t + 1) * 128],
                                 in_=kt_ps, func=AFT.Copy)

        # ---- block j=0 attention, run entirely inside phase 1 ----------
        # scores/exp/den on the spare sc1 bank pair, attn@v into y0 (the 2
        # banks freed by the q_ps/kv_ps shrink), tail + out-projection
        # borrowing the sc1 slot.  All of it fills phase-1 dependency gaps.
        j0_state = {}
        j0_den = [None, None]

        def tail_j0(hp):
            ysb = p1sb.tile([128, 2, BLK], BF16, tag="ysb0", bufs=2,
                            name=f"ysb0_{hp}")
            nc.vector.tensor_copy(out=ysb, in_=j0_state.pop("y0"))
            for hh in range(2):
                h = hp * 2 + hh
                dn_ps = p1ps.tile([1, BLK], F32, tag="sc1_ps", bufs=1,
                                  name=f"dnps0_{h}")
                mm(dn_ps, lhsT=ones_col, rhs=j0_den[hp][:, hh, :],
                   start=True, stop=True)
                rdr = p1sb.tile([1, BLK], BF16, tag="rdr0", bufs=2,
                                name=f"rdr0_{h}")
                nc.vector.reciprocal(rdr, dn_ps)
                rdb_ps = p1ps.tile([128, BLK], F32, tag="sc1_ps", bufs=1,
                                   name=f"rdbps0_{h}")
                mm(rdb_ps, lhsT=ones_row, rhs=rdr, start=True, stop=True)
                nc.vector.tensor_mul(yT_sb[:, h, 0:BLK], ysb[:, hh, :],
                                     rdb_ps)

        def outproj_j0(oc):
            osl = slice(oc * 128, (oc + 1) * 128)
            # alternate between the two spare-bank tags so two
            # out-projection chains overlap
            ops_ = p1ps.tile([128, BLK], F32,
                             tag=("sc1_ps" if oc % 2 else "y0_ps"), bufs=1,
                             name=f"ops0_{oc}")
            for c in range(NH):
                mm(ops_, lhsT=wo_sb[:, c, osl], rhs=yT_sb[:, c, 0:BLK],
                   start=(c == 0), stop=(c == NH - 1))
            oc_sb = p1sb.tile([128, BLK], BF16, tag="oc0_sb", bufs=4,
                              name=f"ocsb0_{oc}")
            if oc % 2 == 0:
                nc.scalar.activation(out=oc_sb, in_=ops_, func=AFT.Copy)
            else:
                nc.vector.tensor_copy(out=oc_sb, in_=ops_)
            nc.sync.dma_start(out=outT[osl, 0:BLK], in_=oc_sb)

        def j0_unit(u):
            hp, t = divmod(u, 4)
            m = t * 128
            tsl = slice(t * 128, (t + 1) * 128)
            sc2 = p1ps.tile([128, 2, BLK], F32, tag="sc1_ps", bufs=1,
                            name=f"sc1_{u}")
            et = p1sb.tile([128, 2, BLK], BF16, tag="et0", bufs=3,
                           name=f"et0_{u}")
            for hh in range(2):
                h = hp * 2 + hh
                mm(sc2[:, hh, m:BLK], lhsT=kT_sb[:, tsl],
                   rhs=qT_sb[:, h, m:BLK], start=True, stop=True)
            nc.scalar.activation(out=et[:, :, m:BLK], in_=sc2[:, :, m:BLK],
                                 func=AFT.Exp, scale=rstdk_sb[:, t:t + 1])
            nc.gpsimd.tensor_mul(
                et[:, :, m:m + 128], et[:, :, m:m + 128],
                tri[:, None, :].broadcast_to([128, 2, 128]))
            if t == 0:
                den = p1sb.tile([128, 2, BLK], BF16, tag="den0", bufs=2,
                                name=f"den0_{hp}")
                j0_den[hp] = den
                nc.vector.tensor_copy(out=den, in_=et)
                j0_state["y0"] = p1ps.tile([128, 2, BLK], F32, tag="y0_ps",
                                           bufs=1, name=f"y0_{hp}")
            else:
                nc.vector.tensor_add(j0_den[hp][:, :, m:BLK],
                                     j0_den[hp][:, :, m:BLK],
                                     et[:, :, m:BLK])
            y0 = j0_state["y0"]
            for hh in range(2):
                mm(y0[:, hh, m:BLK], lhsT=kv_sb[:, t, HD:2 * HD],
                   rhs=et[:, hh, m:BLK], start=(t == 0), stop=(t == 3))
            if t == 3:
                tail_j0(hp)

        j0q = ([(lambda u=u: j0_unit(u)) for u in range(8)] +
               [(lambda oc=oc: outproj_j0(oc)) for oc in range(8)])
        for b in range(NB):
            sl = slice(b * BLK, (b + 1) * BLK)
            xts = []
            for di in range(NDC):
                xt = p1sb.tile([128, BLK], BF16, tag="xt", bufs=16,
                               name=f"xt_{b}_{di}")
                nc.sync.dma_start(out=xt,
                                  in_=xT[di * 128:(di + 1) * 128, sl])
                xts.append(xt)
            if b == 0:
                for di in range(NDC):
                    nc.sync.dma_start(out=wkv_sb[:, di, :],
                                      in_=wkv[:, di, :])
                # wo isn't needed until the first out-projection; issuing
                # it late (split in 4 so transfers parallelize across DMA
                # engines) keeps startup DMA bandwidth for x and wkv.
                for c in range(NH):
                    nc.sync.dma_start(out=wo_sb[:, c, :], in_=wo[:, c, :])
            for half in range(2):
                i0 = b * 4 + half * 2
                qps2 = [p1ps.tile([128, BLK], F32, tag="q_ps", bufs=3,
                                  name=f"qps_{b}_{half}_{x}")
                        for x in range(2)]
                # both x-chunks' kv accumulators packed into one bank
                kvp = p1ps.tile([128, 2, 2 * HD], F32, tag="kv_ps", bufs=1,
                                name=f"kvp_{b}_{half}")
                # kv groups must be contiguous per x (one pending PSUM
                # accumulation group per tile), so kv x=0 runs with q x=0,
                # then kv x=1 with q x=1
                for x in range(2):
                    ii = half * 2 + x
                    for di in range(NDC):
                        st, sp = di == 0, di == NDC - 1
                        xsl = xts[di][:, ii * 128:(ii + 1) * 128]
                        mm(qps2[x], lhsT=xsl, rhs=wq_sb[:, di, :],
                           start=st, stop=sp)
                        mm(kvp[:, x, :], lhsT=xsl, rhs=wkv_sb[:, di, :],
                           start=st, stop=sp)
                # one DVE copy evacuates both kv tiles and frees the bank
                nc.vector.tensor_copy(out=kv_sb[:, i0:i0 + 2, :], in_=kvp)
                for x in range(2):
                    proc_q(i0 + x, qps2[x])
                    proc_k(i0 + x)
                # j=0 attention + out-projection fill phase-1 gaps
                n_u = 0 if b == 0 else (2 if b < 3 else 4)
                for _ in range(n_u):
                    if j0q:
                        j0q.pop(0)()
        while j0q:
            j0q.pop(0)()

    # ---- phase 2+3: attention + per-block output projection ---------------
    # PSUM (8 banks): sc_ps [128,2,512] x2 (4 banks) + y_ps [128,2,512] x2
    # (4 banks); tail dn/rdb and out-projection tiles borrow sc_ps slots.
    with tc.tile_pool(name="p2_psum", bufs=1, space="PSUM") as p2ps, \
         tc.tile_pool(name="p2_sbuf", bufs=1) as p2sb:

        def outproj(j, oc):
            def thunk():
                jsl = slice(j * BLK, (j + 1) * BLK)
                osl = slice(oc * 128, (oc + 1) * 128)
                ops_ = p2ps.tile([128, BLK], F32, tag="aux_ps", bufs=2,
                                 name=f"ops_{j}_{oc}")
                for c in range(NH):
                    mm(ops_, lhsT=wo_sb[:, c, osl], rhs=yT_sb[:, c, jsl],
                       start=(c == 0), stop=(c == NH - 1))
                oc_sb = p2sb.tile([128, BLK], BF16, tag="oc_sb", bufs=6,
                                  name=f"ocsb_{j}_{oc}")
                if oc % 2 == 0:
                    nc.scalar.activation(out=oc_sb, in_=ops_, func=AFT.Copy)
                else:
                    nc.vector.tensor_copy(out=oc_sb, in_=ops_)
                nc.sync.dma_start(out=outT[osl, jsl], in_=oc_sb)
            return thunk

        def tail2(j, hp, yps, den_e, den_o, jsl):
            # softmax tail for this pass's two heads.  One DVE copy frees
            # the y PSUM bank immediately (the next pass's attn@v waits on
            # it); normalization then reads SBUF x rdb-PSUM (one PSUM input).
            ysb = p2sb.tile([128, 2, BLK], BF16, tag="ysb", bufs=2,
                            name=f"ysb_{j}_{hp}")
            nc.vector.tensor_copy(out=ysb, in_=yps)
            for hh in range(2):
                h = hp * 2 + hh
                dn_ps = p2ps.tile([1, BLK], F32, tag="aux_ps", bufs=2,
                                  name=f"dnps_{j}_{h}")
                mm(dn_ps, lhsT=ones_col, rhs=den_e[:, hh, :],
                   start=True, stop=(den_o is None))
                if den_o is not None:
                    mm(dn_ps, lhsT=ones_col, rhs=den_o[:, hh, :],
                       start=False, stop=True)
                rdr = p2sb.tile([1, BLK], BF16, tag="rdr", bufs=4,
                                name=f"rdr_{j}_{h}")
                nc.vector.reciprocal(rdr, dn_ps)
                rdb_ps = p2ps.tile([128, BLK], F32, tag="aux_ps",
                                   bufs=2, name=f"rdbps_{j}_{h}")
                mm(rdb_ps, lhsT=ones_row, rhs=rdr, start=True, stop=True)
                nc.vector.tensor_mul(yT_sb[:, h, jsl], ysb[:, hh, :],
                                     rdb_ps)

        pending = []
        for j in range(1, NB):
            n_t = 4 * j + 4
            jsl = slice(j * BLK, (j + 1) * BLK)
            work, wi = pending, 0
            for hp in range(2):          # head-pair pass (heads 2hp, 2hp+1)
                yps = p2ps.tile([128, 2, BLK], F32, tag="y_ps", bufs=1,
                                name=f"yps_{j}_{hp}")
                den_e = p2sb.tile([128, 2, BLK], BF16, tag="den_e", bufs=2,
                                  name=f"dene_{j}_{hp}")
                den_o = p2sb.tile([128, 2, BLK], BF16, tag="den_o", bufs=2,
                                  name=f"deno_{j}_{hp}")
                for t in range(n_t):
                    diag = t >= 4 * j
                    m = (t - 4 * j) * 128 if diag else 0
                    tsl = slice(t * 128, (t + 1) * 128)
                    sc2 = p2ps.tile([128, 2, BLK], F32, tag="sc_ps",
                                    bufs=2, name=f"sc2_{j}_{hp}_{t}")
                    et2 = p2sb.tile([128, 2, BLK], BF16, tag="expt",
                                    bufs=6, name=f"expt_{j}_{hp}_{t}")
                    for hh in range(2):
                        h = hp * 2 + hh
                        mm(sc2[:, hh, m:BLK], lhsT=kT_sb[:, tsl],
                           rhs=qT_sb[:, h, j * BLK + m:(j + 1) * BLK],
                           start=True, stop=True)
                    nc.scalar.activation(out=et2[:, :, m:BLK],
                                         in_=sc2[:, :, m:BLK], func=AFT.Exp,
                                         scale=rstdk_sb[:, t:t + 1])
                    if diag:
                        nc.gpsimd.tensor_mul(
                            et2[:, :, m:m + 128], et2[:, :, m:m + 128],
                            tri[:, None, :].broadcast_to([128, 2, 128]))
                    if t == 0:
                        nc.vector.tensor_copy(out=den_e, in_=et2)
                    elif j == 0 or t % 2 == 0:
                        nc.vector.tensor_add(den_e[:, :, m:BLK],
                                             den_e[:, :, m:BLK],
                                             et2[:, :, m:BLK])
                    elif t == 1:
                        nc.vector.tensor_copy(out=den_o, in_=et2)
                    else:
                        nc.vector.tensor_add(den_o[:, :, m:BLK],
                                             den_o[:, :, m:BLK],
                                             et2[:, :, m:BLK])
                    for hh in range(2):
                        mm(yps[:, hh, m:BLK], lhsT=kv_sb[:, t, HD:2 * HD],
                           rhs=et2[:, hh, m:BLK],
                           start=(t == 0), stop=(t == n_t - 1))
                    if wi < len(work):
                        work[wi]()
                        wi += 1
                tail2(j, hp, yps, den_e, den_o, jsl)
            while wi < len(work):
                work[wi]()
                wi += 1
            pending = [outproj(j, oc) for oc in range(8)]
        for thunk in pending:
            thunk()

    persist_cm.__exit__(None, None, None)


_NC_CACHE = {}


def _get_nc():
    if "nc" not in _NC_CACHE:
        _NC_CACHE["nc"] = _build_nc()
    return _NC_CACHE["nc"]


def _host_tables():
    pos = np.arange(S, dtype=np.float32)
    inv = (1.0 / (10000.0 ** (np.arange(0, RD, 2, dtype=np.float32) / RD)))
    fr = np.outer(pos, inv).astype(np.float32)          # [S, 32]
    cos, sin = np.cos(fr), np.sin(fr)
    tile128 = lambda a: np.ascontiguousarray(
        a.reshape(NT, 128, RH).transpose(1, 0, 2)).astype(BFNP)
    return tile128(cos), tile128(sin), tile128(-sin)


def core_in_map(x, w_q, w_k, w_v, w_o, q_gain, core):
    """Host-side per-core input prep: shard + transpose + bf16 convert."""
    cosq, sinq, nsinq = _host_tables()
    b, g = divmod(core, 2)
    cols = slice(g * NH * HD, (g + 1) * NH * HD)

    def wtile(wT, chunks, width):
        # [chunks*128, width] -> [128, chunks, width] bf16
        return np.ascontiguousarray(
            wT.reshape(chunks, 128, width).transpose(1, 0, 2)).astype(BFNP)

    xTc = np.ascontiguousarray(x[b].T).astype(BFNP)             # [D, S]
    wq_t = wtile(np.ascontiguousarray(w_q[cols, :].T), NDC, NH * HD)
    wkv_t = wtile(np.ascontiguousarray(np.concatenate(
        [w_k[g * HD:(g + 1) * HD, :].T, w_v[g * HD:(g + 1) * HD, :].T],
        axis=1)), NDC, 2 * HD)
    wo_t = wtile(np.ascontiguousarray(w_o[:, cols].T), NH, D)
    qsc_h = (q_gain[g * NH:(g + 1) * NH] *
             np.float32(HD ** -0.5)).astype(np.float32).reshape(1, NH)
    return dict(xT=xTc, wq=wq_t, wkv=wkv_t, wo=wo_t,
                cosq=cosq, sinq=sinq, nsinq=nsinq, qsc=qsc_h)


def kernel(x, w_q, w_k, w_v, w_o, q_gain):
    x = np.asarray(x, dtype=np.float32)
    w_q = np.asarray(w_q, dtype=np.float32)
    w_k = np.asarray(w_k, dtype=np.float32)
    w_v = np.asarray(w_v, dtype=np.float32)
    w_o = np.asarray(w_o, dtype=np.float32)
    q_gain = np.asarray(q_gain, dtype=np.float32)

    nc = _get_nc()
    in_maps = [core_in_map(x, w_q, w_k, w_v, w_o, q_gain, core)
               for core in range(8)]
    res = bass_utils.run_bass_kernel_spmd(nc, in_maps,
                                          core_ids=list(range(8)))
    out = np.empty((B, S, D), dtype=np.float32)
    for b in range(B):
        p0 = res.results[2 * b]["outT"].astype(np.float32)
        p1 = res.results[2 * b + 1]["outT"].astype(np.float32)
        out[b] = (p0 + p1).T
    return out


# revision 35
# speedup vs baseline: 3.4899x; 2.6356x over previous
"""Trainium2 Bass kernel for CausalSelfAttention (GQA + RMSNorm + partial RoPE).

Sharding: 8 cores = (batch b in 0..3) x (kv-head group g in 0..1).
Each core computes the full attention for its (b, g) slice and the partial
output projection over its head columns; the host sums the two partials per
batch and transposes back ([o, s] -> [s, o]).

All DRAM traffic and SBUF operands are bf16 (host pre-converts); PSUM
accumulation stays fp32.  Key structure per core:
  - Phase 1: QKV projections from xT [d, s] bf16 tiles (weights/x split
    into per-chunk DMAs across the SP and ACT DGE queues for a fast ramp);
    q in [s, o] layout, evacuated with a single ACT copy; RoPE runs on the
    raw q in parallel with the rstd chain and the rstd*gain scale lands in
    two Pool multiplies; the 4 head transposes land bank-packed in one PSUM
    tile, evacuated with one DVE copy to qT [hd, s].  k+v evacuate together
    (one DVE copy); k's rstd folds into the exp() scale.
  - rstd = exp(-0.5*ln(mean+eps)) on ACT keeps every activation in the
    single natural_log_exp table set -> zero table reloads (the chooser is
    pinned via get_activation_tables).
  - Block j=0's whole attention (scores/exp/den on a spare PSUM bank pair,
    attn@v into 2 freed banks, softmax tail + out-projection) is emitted
    interleaved through phase 1, filling its dependency gaps.
  - Phase 2 (j=1..3): per head-pair pass, scoresT [sk, sq] = kT_t.T @ qT;
    Exp on ACT (bf16 out, scale = rstd_k); causal via matmul range limits
    + one triangular mask multiply per diagonal block; denominator
    accumulated on DVE in bf16 (2x 16-bit mode), partition-reduced with
    accumulating ones-matmuls, reciprocal on DVE, broadcast with a K=1
    matmul.  One DVE copy evacuates the attn@v accumulator early (frees
    the y banks for the next pass); normalization multiplies SBUF x PSUM.
  - The out-projection of block j is queued as thunks and interleaved into
    block j+1's tile loop on a dedicated aux-PSUM tag, so its PE work fills
    the ACT-bound attention stretches; outT is bf16, host sums in fp32.
"""

import sys

for _p in ("/opt/trn_rl_repo",):
    if _p not in sys.path:
        sys.path.insert(0, _p)

import numpy as np
import ml_dtypes

import concourse.bass as bass
import concourse.bacc as bacc
import concourse.mybir as mybir
import concourse.tile as tile
from concourse import bass_utils
from concourse.masks import make_identity

F32 = mybir.dt.float32
BF16 = mybir.dt.bfloat16
BFNP = np.dtype(ml_dtypes.bfloat16)
AFT = mybir.ActivationFunctionType

# Pin the ACT table to natural_log_exp_and_others (exp+ln+copy+square — the
# only funcs this kernel uses) so the table chooser can't ping-pong between
# the exp-only and ln+exp sets, which costs a 1.3us table load each flip.
# Positional ids must be preserved, so other sets are blanked, not removed.
_orig_get_act_tables = bacc.get_activation_tables


def _pinned_act_tables(arch):
    tabs = _orig_get_act_tables(arch)
    return {name: (funcs if name == "natural_log_exp_and_others" else set())
            for name, funcs in tabs.items()}


bacc.get_activation_tables = _pinned_act_tables

B, S, D = 4, 2048, 1024
H, KVH, HD = 8, 2, 128
NH = H // KVH          # q heads per core = 4
RD, RH = 64, 32        # rope dims / half
NB, BLK = 4, 512       # s blocks
NT, TS = 16, 128       # s tiles
NDC = D // 128         # 8 d-chunks
EPS = float(np.finfo(np.float32).eps)


def _build_nc(reps=1):
    nc = bacc.Bacc("TRN2", target_bir_lowering=False, debug=False,
                   enable_asserts=False)

    xT = nc.dram_tensor("xT", (D, S), BF16, kind="ExternalInput").ap()
    wq = nc.dram_tensor("wq", (128, NDC, NH * HD), BF16,
                        kind="ExternalInput").ap()
    wkv = nc.dram_tensor("wkv", (128, NDC, 2 * HD), BF16,
                         kind="ExternalInput").ap()
    wo = nc.dram_tensor("wo", (128, NH, D), BF16, kind="ExternalInput").ap()
    cosq = nc.dram_tensor("cosq", (128, NT, RH), BF16,
                          kind="ExternalInput").ap()
    sinq = nc.dram_tensor("sinq", (128, NT, RH), BF16,
                          kind="ExternalInput").ap()
    nsinq = nc.dram_tensor("nsinq", (128, NT, RH), BF16,
                           kind="ExternalInput").ap()
    qsc = nc.dram_tensor("qsc", (1, NH), F32, kind="ExternalInput").ap()
    outT = nc.dram_tensor("outT", (D, S), BF16, kind="ExternalOutput").ap()

    with tile.TileContext(nc) as tc, \
         nc.allow_low_precision(reason="bf16 attention"):
        for _rep in range(reps):
            _kern(nc, tc, xT, wq, wkv, wo, cosq, sinq, nsinq, qsc, outT)
    nc.compile()
    return nc


def _kern(nc, tc, xT, wq, wkv, wo, cosq, sinq, nsinq, qsc, outT):
    mm = nc.tensor.matmul

    persist_cm = tc.tile_pool(name="persist", bufs=1)
    persist = persist_cm.__enter__()
    # ---- persistent tiles -------------------------------------------------
    wq_sb = persist.tile([128, NDC, NH * HD], BF16, tag="wq_sb", name="wq_sb")
    wkv_sb = persist.tile([128, NDC, 2 * HD], BF16, tag="wkv_sb",
                          name="wkv_sb")
    # Weights go in per-chunk DMAs (so the first QKV matmuls can start
    # before the whole tensors land), split across the two HW DGE queues
    # (SP carries wkv, ACT carries wq + tables) to halve descriptor-gen
    # latency at startup.  The x tiles for b=0 are interleaved on SP below.
    wo_sb = persist.tile([128, NH, D], BF16, tag="wo_sb", name="wo_sb")
    cosq_sb = persist.tile([128, NT, RH], BF16, tag="cosq_sb", name="cosq_sb")
    sinq_sb = persist.tile([128, NT, RH], BF16, tag="sinq_sb", name="sinq_sb")
    nsinq_sb = persist.tile([128, NT, RH], BF16, tag="nsinq_sb",
                            name="nsinq_sb")
    qsc_sb = persist.tile([128, NH], F32, tag="qsc_sb", name="qsc_sb")
    for di in range(NDC):
        nc.scalar.dma_start(out=wq_sb[:, di, :], in_=wq[:, di, :])
    nc.scalar.dma_start(out=cosq_sb, in_=cosq)
    nc.scalar.dma_start(out=sinq_sb, in_=sinq)
    nc.scalar.dma_start(out=nsinq_sb, in_=nsinq)
    nc.scalar.dma_start(out=qsc_sb, in_=qsc.to_broadcast((128, NH)))

    ones_col = persist.tile([128, 1], BF16, tag="ones_col", name="ones_col")
    nc.vector.memset(ones_col, 1.0)
    ones_row = persist.tile([1, 128], BF16, tag="ones_row", name="ones_row")
    nc.vector.memset(ones_row, 1.0)
    eps_col = persist.tile([128, 1], F32, tag="eps_col", name="eps_col")
    nc.vector.memset(eps_col, EPS)
    ident_st = persist.tile([128, 128], F32, tag="ident_st", name="ident_st")
    make_identity(nc, ident_st)
    ident = persist.tile([128, 128], BF16, tag="ident", name="ident")
    nc.vector.tensor_copy(out=ident, in_=ident_st)
    # tri[r, c] = 1.0 if r <= c else 0.0  (causal keep-mask on the diagonal)
    tri_st = persist.tile([128, 128], F32, tag="tri_st", name="tri_st")
    nc.gpsimd.memset(tri_st, 1.0)
    nc.gpsimd.affine_select(
        out=tri_st, in_=tri_st, compare_op=mybir.AluOpType.is_ge, fill=0.0,
        base=0, pattern=[[1, 128]], channel_multiplier=-1)
    tri = persist.tile([128, 128], BF16, tag="tri", name="tri")
    nc.vector.tensor_copy(out=tri, in_=tri_st)

    qT_sb = persist.tile([128, NH, S], BF16, tag="qT_sb", name="qT_sb")
    kT_sb = persist.tile([128, S], BF16, tag="kT_sb", name="kT_sb")
    kv_sb = persist.tile([128, NT, 2 * HD], BF16, tag="kv_sb", name="kv_sb")
    rstdk_sb = persist.tile([128, NT], F32, tag="rstdk_sb", name="rstdk_sb")
    yT_sb = persist.tile([128, NH, S], BF16, tag="yT_sb", name="yT_sb")

    # ---- phase 1: projections + norm + rope + transposes ------------------
    # PSUM budget (8 banks): q_ps 4 + kv_ps 2 + misc_ps 2.
    with tc.tile_pool(name="p1_psum", bufs=1, space="PSUM") as p1ps, \
         tc.tile_pool(name="p1_sbuf", bufs=1) as p1sb:

        def proc_q(i, qp):
            # qp: PSUM [128, 512] f32 = q rows for s-tile i, 4 heads x hd.
            # Single ACT copy evacuates PSUM; everything else reads bf16 SBUF.
            qraw = p1sb.tile([128, BLK], BF16, tag="qraw", bufs=3,
                             name=f"qraw_{i}")
            nc.scalar.activation(out=qraw, in_=qp, func=AFT.Copy)
            qrv = qraw.rearrange("p (h f) -> p h f", h=NH)
            sq = p1sb.tile([128, BLK], BF16, tag="sq", bufs=3,
                           name=f"sq_{i}")
            nc.scalar.activation(out=sq, in_=qraw, func=AFT.Square)
            sumsq = p1sb.tile([128, NH], F32, tag="sumsq", bufs=3,
                              name=f"sumsq_{i}")
            nc.vector.tensor_reduce(
                out=sumsq, in_=sq.rearrange("p (h f) -> p h f", h=NH),
                axis=mybir.AxisListType.X, op=mybir.AluOpType.add)
            # rstd = exp(-0.5*ln(mean+eps)): stays in the exp/ln ACT table
            # set, so the whole kernel never reloads activation tables
            ql = p1sb.tile([128, NH], F32, tag="ql", bufs=3,
                           name=f"ql_{i}")
            nc.scalar.activation(out=ql, in_=sumsq, func=AFT.Ln,
                                 bias=eps_col, scale=1.0 / HD)
            rstd = p1sb.tile([128, NH], F32, tag="rstd", bufs=3,
                             name=f"rstd_{i}")
            nc.scalar.activation(out=rstd, in_=ql, func=AFT.Exp, scale=-0.5)
            rsc = p1sb.tile([128, NH], F32, tag="rsc", bufs=3,
                            name=f"rsc_{i}")
            nc.vector.tensor_mul(rsc, rstd, qsc_sb)

            # RoPE on the raw q (independent of the rstd chain); the
            # rstd*gain scale lands in the final two Pool multiplies only
            cos4 = cosq_sb[:, i:i + 1, None, :].broadcast_to(
                [128, NH, 2, RH])
            sin3 = sinq_sb[:, i:i + 1, :].broadcast_to([128, NH, RH])
            nsin3 = nsinq_sb[:, i:i + 1, :].broadcast_to([128, NH, RH])
            tcq = p1sb.tile([128, NH, RD], BF16, tag="tcq", bufs=3,
                            name=f"tcq_{i}")
            nc.vector.tensor_mul(
                tcq.rearrange("p h (two f) -> p h two f", two=2),
                qrv[:, :, 0:RD].rearrange("p h (two f) -> p h two f", two=2),
                cos4)
            tsq = p1sb.tile([128, NH, RD], BF16, tag="tsq", bufs=3,
                            name=f"tsq_{i}")
            nc.vector.tensor_mul(tsq[:, :, 0:RH], qrv[:, :, RH:RD], sin3)
            nc.vector.tensor_mul(tsq[:, :, RH:RD], qrv[:, :, 0:RH], nsin3)
            nc.vector.tensor_add(tcq, tcq, tsq)
            qn = p1sb.tile([128, BLK], BF16, tag="qn", bufs=3,
                           name=f"qn_{i}")
            qnv = qn.rearrange("p (h f) -> p h f", h=NH)
            rsc_b = rsc[:, :, None].broadcast_to([128, NH, RD])
            nc.gpsimd.tensor_mul(qnv[:, :, 0:RD], tcq, rsc_b)
            nc.gpsimd.tensor_mul(qnv[:, :, RD:HD], qrv[:, :, RD:HD], rsc_b)
            qt_ps = p1ps.tile([128, NH, 128], BF16, tag="kv_ps", bufs=1,
                              name=f"qtps_{i}")
            for h in range(NH):
                nc.tensor.transpose(qt_ps[:, h, :],
                                    qn[:, h * 128:(h + 1) * 128], ident)
            nc.vector.tensor_copy(
                out=qT_sb[:, :, i * 128:(i + 1) * 128], in_=qt_ps)

        def proc_k(t, late=False):
            v_eng = nc.gpsimd if late else nc.vector
            # reads k half of kv_sb[:, t, :] (bf16 SBUF)
            kn = kv_sb[:, t, 0:HD]
            ksq = p1sb.tile([128, HD], BF16, tag="ksq", bufs=3,
                            name=f"ksq_{t}")
            ksum = p1sb.tile([128, 1], F32, tag="ksum", bufs=3,
                             name=f"ksum_{t}")
            nc.scalar.activation(out=ksq, in_=kn, func=AFT.Square,
                                 accum_out=ksum)
            kl = p1sb.tile([128, 1], F32, tag="kl", bufs=3,
                           name=f"kl_{t}")
            nc.scalar.activation(out=kl, in_=ksum, func=AFT.Ln,
                                 bias=eps_col, scale=1.0 / HD)
            nc.scalar.activation(out=rstdk_sb[:, t:t + 1], in_=kl,
                                 func=AFT.Exp, scale=-0.5)
            cos2 = cosq_sb[:, t:t + 1, :].broadcast_to([128, 2, RH])
            tck = p1sb.tile([128, RD], BF16, tag="tck", bufs=3,
                            name=f"tck_{t}")
            v_eng.tensor_mul(
                tck.rearrange("p (two f) -> p two f", two=2),
                kn[:, 0:RD].rearrange("p (two f) -> p two f", two=2), cos2)
            tsk = p1sb.tile([128, RD], BF16, tag="tsk", bufs=3,
                            name=f"tsk_{t}")
            v_eng.tensor_mul(tsk[:, 0:RH], kn[:, RH:RD],
                                 sinq_sb[:, t, :])
            v_eng.tensor_mul(tsk[:, RH:RD], kn[:, 0:RH],
                                 nsinq_sb[:, t, :])
            kst = p1sb.tile([128, HD], BF16, tag="kst", bufs=3,
                            name=f"kst_{t}")
            v_eng.tensor_add(kst[:, 0:RD], tck, tsk)
            v_eng.tensor_copy(out=kst[:, RD:HD], in_=kn[:, RD:HD])
            kt_ps = p1ps.tile([128, 128], BF16, tag="kv_ps", bufs=1,
                              name=f"ktps_{t}")
            nc.tensor.transpose(kt_ps, kst, ident)
            nc.scalar.activation(out=kT_sb[:, t * 128:(t + 1) * 128],
                                 in_=kt_ps, func=AFT.Copy)

        # ---- block j=0 attention, run entirely inside phase 1 ----------
        # scores/exp/den on the spare sc1 bank pair, attn@v into y0 (the 2
        # banks freed by the q_ps/kv_ps shrink), tail + out-projection
        # borrowing the sc1 slot.  All of it fills phase-1 dependency gaps.
        j0_state = {}
        j0_den = [None, None]

        def tail_j0(hp):
            ysb = p1sb.tile([128, 2, BLK], BF16, tag="ysb0", bufs=2,
                            name=f"ysb0_{hp}")
            nc.vector.tensor_copy(out=ysb, in_=j0_state.pop("y0"))
            for hh in range(2):
                h = hp * 2 + hh
                dn_ps = p1ps.tile([1, BLK], F32, tag="sc1_ps", bufs=1,
                                  name=f"dnps0_{h}")
                mm(dn_ps, lhsT=ones_col, rhs=j0_den[hp][:, hh, :],
                   start=True, stop=True)
                rdr = p1sb.tile([1, BLK], BF16, tag="rdr0", bufs=2,
                                name=f"rdr0_{h}")
                nc.vector.reciprocal(rdr, dn_ps)
                rdb_ps = p1ps.tile([128, BLK], F32, tag="sc1_ps", bufs=1,
                                   name=f"rdbps0_{h}")
                mm(rdb_ps, lhsT=ones_row, rhs=rdr, start=True, stop=True)
                nc.vector.tensor_mul(yT_sb[:, h, 0:BLK], ysb[:, hh, :],
                                     rdb_ps)

        def outproj_j0(oc):
            osl = slice(oc * 128, (oc + 1) * 128)
            # alternate between the two spare-bank tags so two
            # out-projection chains overlap
            ops_ = p1ps.tile([128, BLK], F32,
                             tag=("sc1_ps" if oc % 2 else "y0_ps"), bufs=1,
                             name=f"ops0_{oc}")
            for c in range(NH):
                mm(ops_, lhsT=wo_sb[:, c, osl], rhs=yT_sb[:, c, 0:BLK],
                   start=(c == 0), stop=(c == NH - 1))
            oc_sb = p1sb.tile([128, BLK], BF16, tag="oc0_sb", bufs=4,
                              name=f"ocsb0_{oc}")
            if oc % 2 == 0:
                nc.scalar.activation(out=oc_sb, in_=ops_, func=AFT.Copy)
            else:
                nc.vector.tensor_copy(out=oc_sb, in_=ops_)
            nc.sync.dma_start(out=outT[osl, 0:BLK], in_=oc_sb)

        def j0_unit(u):
            hp, t = divmod(u, 4)
            m = t * 128
            tsl = slice(t * 128, (t + 1) * 128)
            sc2 = p1ps.tile([128, 2, BLK], F32, tag="sc1_ps", bufs=1,
                            name=f"sc1_{u}")
            et = p1sb.tile([128, 2, BLK], BF16, tag="et0", bufs=3,
                           name=f"et0_{u}")
            for hh in range(2):
                h = hp * 2 + hh
                mm(sc2[:, hh, m:BLK], lhsT=kT_sb[:, tsl],
                   rhs=qT_sb[:, h, m:BLK], start=True, stop=True)
            nc.scalar.activation(out=et[:, :, m:BLK], in_=sc2[:, :, m:BLK],
                                 func=AFT.Exp, scale=rstdk_sb[:, t:t + 1])
            nc.gpsimd.tensor_mul(
                et[:, :, m:m + 128], et[:, :, m:m + 128],
                tri[:, None, :].broadcast_to([128, 2, 128]))
            if t == 0:
                den = p1sb.tile([128, 2, BLK], BF16, tag="den0", bufs=2,
                                name=f"den0_{hp}")
                j0_den[hp] = den
                nc.vector.tensor_copy(out=den, in_=et)
                j0_state["y0"] = p1ps.tile([128, 2, BLK], F32, tag="y0_ps",
                                           bufs=1, name=f"y0_{hp}")
            else:
                nc.vector.tensor_add(j0_den[hp][:, :, m:BLK],
                                     j0_den[hp][:, :, m:BLK],
                                     et[:, :, m:BLK])
            y0 = j0_state["y0"]
            for hh in range(2):
                mm(y0[:, hh, m:BLK], lhsT=kv_sb[:, t, HD:2 * HD],
                   rhs=et[:, hh, m:BLK], start=(t == 0), stop=(t == 3))
            if t == 3:
                tail_j0(hp)

        j0q = ([(lambda u=u: j0_unit(u)) for u in range(8)] +
               [(lambda oc=oc: outproj_j0(oc)) for oc in range(8)])
        for b in range(NB):
            sl = slice(b * BLK, (b + 1) * BLK)
            xts = []
            for di in range(NDC):
                xt = p1sb.tile([128, BLK], BF16, tag="xt", bufs=16,
                               name=f"xt_{b}_{di}")
                nc.sync.dma_start(out=xt,
                                  in_=xT[di * 128:(di + 1) * 128, sl])
                xts.append(xt)
            if b == 0:
                for di in range(NDC):
                    nc.sync.dma_start(out=wkv_sb[:, di, :],
                                      in_=wkv[:, di, :])
                # wo isn't needed until the first out-projection; issuing
                # it late (split in 4 so transfers parallelize across DMA
                # engines) keeps startup DMA bandwidth for x and wkv.
                for c in range(NH):
                    nc.sync.dma_start(out=wo_sb[:, c, :], in_=wo[:, c, :])
            for half in range(2):
                i0 = b * 4 + half * 2
                qps2 = [p1ps.tile([128, BLK], F32, tag="q_ps", bufs=3,
                                  name=f"qps_{b}_{half}_{x}")
                        for x in range(2)]
                # both x-chunks' kv accumulators packed into one bank
                kvp = p1ps.tile([128, 2, 2 * HD], F32, tag="kv_ps", bufs=1,
                                name=f"kvp_{b}_{half}")
                # kv groups must be contiguous per x (one pending PSUM
                # accumulation group per tile), so kv x=0 runs with q x=0,
                # then kv x=1 with q x=1
                for x in range(2):
                    ii = half * 2 + x
                    for di in range(NDC):
                        st, sp = di == 0, di == NDC - 1
                        xsl = xts[di][:, ii * 128:(ii + 1) * 128]
                        mm(qps2[x], lhsT=xsl, rhs=wq_sb[:, di, :],
                           start=st, stop=sp)
                        mm(kvp[:, x, :], lhsT=xsl, rhs=wkv_sb[:, di, :],
                           start=st, stop=sp)
                # one DVE copy evacuates both kv tiles and frees the bank
                nc.vector.tensor_copy(out=kv_sb[:, i0:i0 + 2, :], in_=kvp)
                for x in range(2):
                    proc_q(i0 + x, qps2[x])
                    proc_k(i0 + x)
                # j=0 attention + out-projection fill phase-1 gaps
                n_u = 0 if b == 0 else (2 if b < 3 else 4)
                for _ in range(n_u):
                    if j0q:
                        j0q.pop(0)()
        while j0q:
            j0q.pop(0)()

    # ---- phase 2+3: attention + per-block output projection ---------------
    # PSUM (8 banks): sc_ps [128,2,512] x2 (4 banks) + y_ps [128,2,512] x2
    # (4 banks); tail dn/rdb and out-projection tiles borrow sc_ps slots.
    with tc.tile_pool(name="p2_psum", bufs=1, space="PSUM") as p2ps, \
         tc.tile_pool(name="p2_sbuf", bufs=1) as p2sb:

        def outproj(j, oc):
            def thunk():
                jsl = slice(j * BLK, (j + 1) * BLK)
                osl = slice(oc * 128, (oc + 1) * 128)
                ops_ = p2ps.tile([128, BLK], F32, tag="aux_ps", bufs=2,
                                 name=f"ops_{j}_{oc}")
                for c in range(NH):
                    mm(ops_, lhsT=wo_sb[:, c, osl], rhs=yT_sb[:, c, jsl],
                       start=(c == 0), stop=(c == NH - 1))
                oc_sb = p2sb.tile([128, BLK], BF16, tag="oc_sb", bufs=6,
                                  name=f"ocsb_{j}_{oc}")
                if oc % 2 == 0:
                    nc.scalar.activation(out=oc_sb, in_=ops_, func=AFT.Copy)
                else:
                    nc.vector.tensor_copy(out=oc_sb, in_=ops_)
                nc.sync.dma_start(out=outT[osl, jsl], in_=oc_sb)
            return thunk

        def tail2(j, hp, yps, den_e, den_o, jsl):
            # softmax tail for this pass's two heads.  One DVE copy frees
            # the y PSUM bank immediately (the next pass's attn@v waits on
            # it); normalization then reads SBUF x rdb-PSUM (one PSUM input).
            ysb = p2sb.tile([128, 2, BLK], BF16, tag="ysb", bufs=2,
                            name=f"ysb_{j}_{hp}")
            nc.vector.tensor_copy(out=ysb, in_=yps)
            for hh in range(2):
                h = hp * 2 + hh
                dn_ps = p2ps.tile([1, BLK], F32, tag="aux_ps", bufs=2,
                                  name=f"dnps_{j}_{h}")
                mm(dn_ps, lhsT=ones_col, rhs=den_e[:, hh, :],
                   start=True, stop=(den_o is None))
                if den_o is not None:
                    mm(dn_ps, lhsT=ones_col, rhs=den_o[:, hh, :],
                       start=False, stop=True)
                rdr = p2sb.tile([1, BLK], BF16, tag="rdr", bufs=4,
                                name=f"rdr_{j}_{h}")
                nc.vector.reciprocal(rdr, dn_ps)
                rdb_ps = p2ps.tile([128, BLK], F32, tag="aux_ps",
                                   bufs=2, name=f"rdbps_{j}_{h}")
                mm(rdb_ps, lhsT=ones_row, rhs=rdr, start=True, stop=True)
                nc.vector.tensor_mul(yT_sb[:, h, jsl], ysb[:, hh, :],
                                     rdb_ps)

        pending = []
        for j in range(1, NB):
            n_t = 4 * j + 4
            jsl = slice(j * BLK, (j + 1) * BLK)
            work, wi = pending, 0
            for hp in range(2):          # head-pair pass (heads 2hp, 2hp+1)
                yps = p2ps.tile([128, 2, BLK], F32, tag="y_ps", bufs=1,
                                name=f"yps_{j}_{hp}")
                den_e = p2sb.tile([128, 2, BLK], BF16, tag="den_e", bufs=2,
                                  name=f"dene_{j}_{hp}")
                den_o = p2sb.tile([128, 2, BLK], BF16, tag="den_o", bufs=2,
                                  name=f"deno_{j}_{hp}")
                for t in range(n_t):
                    diag = t >= 4 * j
                    m = (t - 4 * j) * 128 if diag else 0
                    tsl = slice(t * 128, (t + 1) * 128)
                    sc2 = p2ps.tile([128, 2, BLK], F32, tag="sc_ps",
                                    bufs=2, name=f"sc2_{j}_{hp}_{t}")
                    et2 = p2sb.tile([128, 2, BLK], BF16, tag="expt",
                                    bufs=6, name=f"expt_{j}_{hp}_{t}")
                    for hh in range(2):
                        h = hp * 2 + hh
                        mm(sc2[:, hh, m:BLK], lhsT=kT_sb[:, tsl],
                           rhs=qT_sb[:, h, j * BLK + m:(j + 1) * BLK],
                           start=True, stop=True)
                    nc.scalar.activation(out=et2[:, :, m:BLK],
                                         in_=sc2[:, :, m:BLK], func=AFT.Exp,
                                         scale=rstdk_sb[:, t:t + 1])
                    if diag:
                        nc.gpsimd.tensor_mul(
                            et2[:, :, m:m + 128], et2[:, :, m:m + 128],
                            tri[:, None, :].broadcast_to([128, 2, 128]))
                    if t == 0:
                        nc.vector.tensor_copy(out=den_e, in_=et2)
                    elif j == 0 or t % 2 == 0:
                        nc.vector.tensor_add(den_e[:, :, m:BLK],
                                             den_e[:, :, m:BLK],
                                             et2[:, :, m:BLK])
                    elif t == 1:
                        nc.vector.tensor_copy(out=den_o, in_=et2)
                    else:
                        nc.vector.tensor_add(den_o[:, :, m:BLK],
                                             den_o[:, :, m:BLK],
                                             et2[:, :, m:BLK])
                    for hh in range(2):
                        mm(yps[:, hh, m:BLK], lhsT=kv_sb[:, t, HD:2 * HD],
                           rhs=et2[:, hh, m:BLK],
                           start=(t == 0), stop=(t == n_t - 1))
                    if wi < len(work):
                        work[wi]()
                        wi += 1
                tail2(j, hp, yps, den_e, den_o, jsl)
            while wi < len(work):
                work[wi]()
                wi += 1
            pending = [outproj(j, oc) for oc in range(8)]
        for thunk in pending:
            thunk()

    persist_cm.__exit__(None, None, None)


_NC_CACHE = {}


def _get_nc():
    if "nc" not in _NC_CACHE:
        _NC_CACHE["nc"] = _build_nc()
    return _NC_CACHE["nc"]


def _host_tables():
    pos = np.arange(S, dtype=np.float32)
    inv = (1.0 / (10000.0 ** (np.arange(0, RD, 2, dtype=np.float32) / RD)))
    fr = np.outer(pos, inv).astype(np.float32)          # [S, 32]
    cos, sin = np.cos(fr), np.sin(fr)
    tile128 = lambda a: np.ascontiguousarray(
        a.reshape(NT, 128, RH).transpose(1, 0, 2)).astype(BFNP)
    return tile128(cos), tile128(sin), tile128(-sin)


def core_in_map(x, w_q, w_k, w_v, w_o, q_gain, core):
    """Host-side per-core input prep: shard + transpose + bf16 convert."""
    cosq, sinq, nsinq = _host_tables()
    b, g = divmod(core, 2)
    cols = slice(g * NH * HD, (g + 1) * NH * HD)

    def wtile(wT, chunks, width):
        # [chunks*128, width] -> [128, chunks, width] bf16
        return np.ascontiguousarray(
            wT.reshape(chunks, 128, width).transpose(1, 0, 2)).astype(BFNP)

    xTc = np.ascontiguousarray(x[b].T).astype(BFNP)             # [D, S]
    wq_t = wtile(np.ascontiguousarray(w_q[cols, :].T), NDC, NH * HD)
    wkv_t = wtile(np.ascontiguousarray(np.concatenate(
        [w_k[g * HD:(g + 1) * HD, :].T, w_v[g * HD:(g + 1) * HD, :].T],
        axis=1)), NDC, 2 * HD)
    wo_t = wtile(np.ascontiguousarray(w_o[:, cols].T), NH, D)
    qsc_h = (q_gain[g * NH:(g + 1) * NH] *
             np.float32(HD ** -0.5)).astype(np.float32).reshape(1, NH)
    return dict(xT=xTc, wq=wq_t, wkv=wkv_t, wo=wo_t,
                cosq=cosq, sinq=sinq, nsinq=nsinq, qsc=qsc_h)


def kernel(x, w_q, w_k, w_v, w_o, q_gain):
    x = np.asarray(x, dtype=np.float32)
    w_q = np.asarray(w_q, dtype=np.float32)
    w_k = np.asarray(w_k, dtype=np.float32)
    w_v = np.asarray(w_v, dtype=np.float32)
    w_o = np.asarray(w_o, dtype=np.float32)
    q_gain = np.asarray(q_gain, dtype=np.float32)

    nc = _get_nc()
    in_maps = [core_in_map(x, w_q, w_k, w_v, w_o, q_gain, core)
               for core in range(8)]
    res = bass_utils.run_bass_kernel_spmd(nc, in_maps,
                                          core_ids=list(range(8)))
    out = np.empty((B, S, D), dtype=np.float32)
    for b in range(B):
        p0 = res.results[2 * b]["outT"].astype(np.float32)
        p1 = res.results[2 * b + 1]["outT"].astype(np.float32)
        out[b] = (p0 + p1).T
    return out


# revision 40
# speedup vs baseline: 3.9723x; 1.1383x over previous
"""Trainium2 Bass kernel for CausalSelfAttention (GQA + RMSNorm + partial RoPE).

Sharding: 8 cores = (batch b in 0..3) x (kv-head group g in 0..1).
Each core computes the full attention for its (b, g) slice and the partial
output projection over its head columns; the host sums the two partials per
batch and transposes back ([o, s] -> [s, o]).

All DRAM traffic and SBUF operands are bf16 (host pre-converts); PSUM
accumulation stays fp32.  Key structure per core:
  - Phase 1: QKV projections from xT [d, s] bf16 tiles (weights/x split
    into per-chunk DMAs across the SP and ACT DGE queues for a fast ramp);
    q in [s, o] layout, evacuated with a single ACT copy; RoPE runs on the
    raw q in parallel with the rstd chain and the rstd*gain scale lands in
    two Pool multiplies; the 4 head transposes land bank-packed in one PSUM
    tile, evacuated with one DVE copy to qT [hd, s].  k+v evacuate together
    (one DVE copy); k's rstd folds into the exp() scale.
  - rstd = exp(-0.5*ln(mean+eps)) on ACT keeps every activation in the
    single natural_log_exp table set -> zero table reloads (the chooser is
    pinned via get_activation_tables).
  - Block j=0's whole attention (scores/exp/den on a spare PSUM bank pair,
    attn@v into 2 freed banks, softmax tail + out-projection) is emitted
    interleaved through phase 1, filling its dependency gaps.
  - Phase 2 (j=1..3): per head-pair pass, scoresT [sk, sq] = kT_t.T @ qT;
    Exp on ACT (bf16 out, scale = rstd_k); causal via matmul range limits
    + one triangular mask multiply per diagonal block; denominator
    accumulated on DVE in bf16 (2x 16-bit mode), partition-reduced with
    accumulating ones-matmuls, reciprocal on DVE, broadcast with a K=1
    matmul.  One DVE copy evacuates the attn@v accumulator early (frees
    the y banks for the next pass); normalization multiplies SBUF x PSUM.
  - The out-projection of block j is queued as thunks and interleaved into
    block j+1's tile loop on a dedicated aux-PSUM tag, so its PE work fills
    the ACT-bound attention stretches; outT is bf16, host sums in fp32.
"""

import sys

for _p in ("/opt/trn_rl_repo",):
    if _p not in sys.path:
        sys.path.insert(0, _p)

import numpy as np
import ml_dtypes

import concourse.bass as bass
import concourse.bacc as bacc
import concourse.mybir as mybir
import concourse.tile as tile
from concourse import bass_utils
from concourse.masks import make_identity

F32 = mybir.dt.float32
BF16 = mybir.dt.bfloat16
BFNP = np.dtype(ml_dtypes.bfloat16)
AFT = mybir.ActivationFunctionType

# Pin the ACT table to natural_log_exp_and_others (exp+ln+copy+square — the
# only funcs this kernel uses) so the table chooser can't ping-pong between
# the exp-only and ln+exp sets, which costs a 1.3us table load each flip.
# Positional ids must be preserved, so other sets are blanked, not removed.
_orig_get_act_tables = bacc.get_activation_tables


def _pinned_act_tables(arch):
    tabs = _orig_get_act_tables(arch)
    return {name: (funcs if name == "natural_log_exp_and_others" else set())
            for name, funcs in tabs.items()}


bacc.get_activation_tables = _pinned_act_tables

B, S, D = 4, 2048, 1024
H, KVH, HD = 8, 2, 128
NH = H // KVH          # q heads per core = 4
RD, RH = 64, 32        # rope dims / half
NB, BLK = 4, 512       # s blocks
NT, TS = 16, 128       # s tiles
NDC = D // 128         # 8 d-chunks
EPS = float(np.finfo(np.float32).eps)


def _build_nc(reps=1):
    nc = bacc.Bacc("TRN2", target_bir_lowering=False, debug=False,
                   enable_asserts=False)

    xT = nc.dram_tensor("xT", (D, S), BF16, kind="ExternalInput").ap()
    wq = nc.dram_tensor("wq", (128, NDC, NH * HD), BF16,
                        kind="ExternalInput").ap()
    wkv = nc.dram_tensor("wkv", (128, NDC, 2 * HD), BF16,
                         kind="ExternalInput").ap()
    wo = nc.dram_tensor("wo", (128, NH, D), BF16, kind="ExternalInput").ap()
    cosq = nc.dram_tensor("cosq", (128, NT, RH), BF16,
                          kind="ExternalInput").ap()
    sinq = nc.dram_tensor("sinq", (128, NT, RH), BF16,
                          kind="ExternalInput").ap()
    nsinq = nc.dram_tensor("nsinq", (128, NT, RH), BF16,
                           kind="ExternalInput").ap()
    qsc = nc.dram_tensor("qsc", (1, NH), F32, kind="ExternalInput").ap()
    outT = nc.dram_tensor("outT", (D, S), BF16, kind="ExternalOutput").ap()

    with tile.TileContext(nc) as tc, \
         nc.allow_low_precision(reason="bf16 attention"):
        for _rep in range(reps):
            _kern(nc, tc, xT, wq, wkv, wo, cosq, sinq, nsinq, qsc, outT)
    nc.compile()
    return nc


def _kern(nc, tc, xT, wq, wkv, wo, cosq, sinq, nsinq, qsc, outT):
    mm = nc.tensor.matmul

    persist_cm = tc.tile_pool(name="persist", bufs=1)
    persist = persist_cm.__enter__()
    # ---- persistent tiles -------------------------------------------------
    wq_sb = persist.tile([128, NDC, NH * HD], BF16, tag="wq_sb", name="wq_sb")
    wkv_sb = persist.tile([128, NDC, 2 * HD], BF16, tag="wkv_sb",
                          name="wkv_sb")
    # Weights go in per-chunk DMAs (so the first QKV matmuls can start
    # before the whole tensors land), split across the two HW DGE queues
    # (SP carries wkv, ACT carries wq + tables) to halve descriptor-gen
    # latency at startup.  The x tiles for b=0 are interleaved on SP below.
    wo_sb = persist.tile([128, NH, D], BF16, tag="wo_sb", name="wo_sb")
    cosq_sb = persist.tile([128, NT, RH], BF16, tag="cosq_sb", name="cosq_sb")
    sinq_sb = persist.tile([128, NT, RH], BF16, tag="sinq_sb", name="sinq_sb")
    nsinq_sb = persist.tile([128, NT, RH], BF16, tag="nsinq_sb",
                            name="nsinq_sb")
    qsc_sb = persist.tile([128, NH], F32, tag="qsc_sb", name="qsc_sb")
    for di in range(NDC):
        nc.scalar.dma_start(out=wq_sb[:, di, :], in_=wq[:, di, :])
    nc.scalar.dma_start(out=cosq_sb, in_=cosq)
    nc.scalar.dma_start(out=sinq_sb, in_=sinq)
    nc.scalar.dma_start(out=nsinq_sb, in_=nsinq)
    nc.scalar.dma_start(out=qsc_sb, in_=qsc.to_broadcast((128, NH)))

    ones_col = persist.tile([128, 1], BF16, tag="ones_col", name="ones_col")
    nc.vector.memset(ones_col, 1.0)
    ones_row = persist.tile([1, 128], BF16, tag="ones_row", name="ones_row")
    nc.vector.memset(ones_row, 1.0)
    eps_col = persist.tile([128, 1], F32, tag="eps_col", name="eps_col")
    nc.vector.memset(eps_col, EPS)
    ident_st = persist.tile([128, 128], F32, tag="ident_st", name="ident_st")
    make_identity(nc, ident_st)
    ident = persist.tile([128, 128], BF16, tag="ident", name="ident")
    nc.vector.tensor_copy(out=ident, in_=ident_st)
    # tri[r, c] = 1.0 if r <= c else 0.0  (causal keep-mask on the diagonal)
    tri_st = persist.tile([128, 128], F32, tag="tri_st", name="tri_st")
    nc.gpsimd.memset(tri_st, 1.0)
    nc.gpsimd.affine_select(
        out=tri_st, in_=tri_st, compare_op=mybir.AluOpType.is_ge, fill=0.0,
        base=0, pattern=[[1, 128]], channel_multiplier=-1)
    tri = persist.tile([128, 128], BF16, tag="tri", name="tri")
    nc.vector.tensor_copy(out=tri, in_=tri_st)

    qT_sb = persist.tile([128, NH, S], BF16, tag="qT_sb", name="qT_sb")
    kT_sb = persist.tile([128, S], BF16, tag="kT_sb", name="kT_sb")
    kv_sb = persist.tile([128, NT, 2 * HD], BF16, tag="kv_sb", name="kv_sb")
    rstdk_sb = persist.tile([128, NT], F32, tag="rstdk_sb", name="rstdk_sb")
    yT_sb = persist.tile([128, NH, S], BF16, tag="yT_sb", name="yT_sb")

    # ---- phase 1: projections + norm + rope + transposes ------------------
    # PSUM budget (8 banks): q_ps 4 + kv_ps 2 + misc_ps 2.
    with tc.tile_pool(name="p1_psum", bufs=1, space="PSUM") as p1ps, \
         tc.tile_pool(name="p1_sbuf", bufs=1) as p1sb:

        def proc_q(i, qp):
            # qp: PSUM [128, 512] f32 = q rows for s-tile i, 4 heads x hd.
            # Single ACT copy evacuates PSUM; everything else reads bf16 SBUF.
            qraw = p1sb.tile([128, BLK], BF16, tag="qraw", bufs=3,
                             name=f"qraw_{i}")
            nc.scalar.activation(out=qraw, in_=qp, func=AFT.Copy)
            qrv = qraw.rearrange("p (h f) -> p h f", h=NH)
            sq = p1sb.tile([128, BLK], BF16, tag="sq", bufs=3,
                           name=f"sq_{i}")
            nc.scalar.activation(out=sq, in_=qraw, func=AFT.Square)
            sumsq = p1sb.tile([128, NH], F32, tag="sumsq", bufs=3,
                              name=f"sumsq_{i}")
            nc.vector.tensor_reduce(
                out=sumsq, in_=sq.rearrange("p (h f) -> p h f", h=NH),
                axis=mybir.AxisListType.X, op=mybir.AluOpType.add)
            # rstd = exp(-0.5*ln(mean+eps)): stays in the exp/ln ACT table
            # set, so the whole kernel never reloads activation tables
            ql = p1sb.tile([128, NH], F32, tag="ql", bufs=3,
                           name=f"ql_{i}")
            nc.scalar.activation(out=ql, in_=sumsq, func=AFT.Ln,
                                 bias=eps_col, scale=1.0 / HD)
            rstd = p1sb.tile([128, NH], F32, tag="rstd", bufs=3,
                             name=f"rstd_{i}")
            nc.scalar.activation(out=rstd, in_=ql, func=AFT.Exp, scale=-0.5)
            rsc = p1sb.tile([128, NH], F32, tag="rsc", bufs=3,
                            name=f"rsc_{i}")
            nc.vector.tensor_mul(rsc, rstd, qsc_sb)

            # RoPE on the raw q (independent of the rstd chain); the
            # rstd*gain scale lands in the final two Pool multiplies only
            cos4 = cosq_sb[:, i:i + 1, None, :].broadcast_to(
                [128, NH, 2, RH])
            sin3 = sinq_sb[:, i:i + 1, :].broadcast_to([128, NH, RH])
            nsin3 = nsinq_sb[:, i:i + 1, :].broadcast_to([128, NH, RH])
            tcq = p1sb.tile([128, NH, RD], BF16, tag="tcq", bufs=3,
                            name=f"tcq_{i}")
            nc.vector.tensor_mul(
                tcq.rearrange("p h (two f) -> p h two f", two=2),
                qrv[:, :, 0:RD].rearrange("p h (two f) -> p h two f", two=2),
                cos4)
            tsq = p1sb.tile([128, NH, RD], BF16, tag="tsq", bufs=3,
                            name=f"tsq_{i}")
            nc.vector.tensor_mul(tsq[:, :, 0:RH], qrv[:, :, RH:RD], sin3)
            nc.vector.tensor_mul(tsq[:, :, RH:RD], qrv[:, :, 0:RH], nsin3)
            nc.vector.tensor_add(tcq, tcq, tsq)
            qn = p1sb.tile([128, BLK], BF16, tag="qn", bufs=3,
                           name=f"qn_{i}")
            qnv = qn.rearrange("p (h f) -> p h f", h=NH)
            rsc_b = rsc[:, :, None].broadcast_to([128, NH, RD])
            nc.gpsimd.tensor_mul(qnv[:, :, 0:RD], tcq, rsc_b)
            nc.gpsimd.tensor_mul(qnv[:, :, RD:HD], qrv[:, :, RD:HD], rsc_b)
            qt_ps = p1ps.tile([128, NH, 128], BF16, tag="kv_ps", bufs=1,
                              name=f"qtps_{i}")
            for h in range(NH):
                nc.tensor.transpose(qt_ps[:, h, :],
                                    qn[:, h * 128:(h + 1) * 128], ident)
            nc.vector.tensor_copy(
                out=qT_sb[:, :, i * 128:(i + 1) * 128], in_=qt_ps)

        def proc_k(t, late=False):
            v_eng = nc.gpsimd if late else nc.vector
            # reads k half of kv_sb[:, t, :] (bf16 SBUF)
            kn = kv_sb[:, t, 0:HD]
            ksq = p1sb.tile([128, HD], BF16, tag="ksq", bufs=3,
                            name=f"ksq_{t}")
            ksum = p1sb.tile([128, 1], F32, tag="ksum", bufs=3,
                             name=f"ksum_{t}")
            nc.scalar.activation(out=ksq, in_=kn, func=AFT.Square,
                                 accum_out=ksum)
            kl = p1sb.tile([128, 1], F32, tag="kl", bufs=3,
                           name=f"kl_{t}")
            nc.scalar.activation(out=kl, in_=ksum, func=AFT.Ln,
                                 bias=eps_col, scale=1.0 / HD)
            nc.scalar.activation(out=rstdk_sb[:, t:t + 1], in_=kl,
                                 func=AFT.Exp, scale=-0.5)
            cos2 = cosq_sb[:, t:t + 1, :].broadcast_to([128, 2, RH])
            tck = p1sb.tile([128, RD], BF16, tag="tck", bufs=3,
                            name=f"tck_{t}")
            v_eng.tensor_mul(
                tck.rearrange("p (two f) -> p two f", two=2),
                kn[:, 0:RD].rearrange("p (two f) -> p two f", two=2), cos2)
            tsk = p1sb.tile([128, RD], BF16, tag="tsk", bufs=3,
                            name=f"tsk_{t}")
            v_eng.tensor_mul(tsk[:, 0:RH], kn[:, RH:RD],
                                 sinq_sb[:, t, :])
            v_eng.tensor_mul(tsk[:, RH:RD], kn[:, 0:RH],
                                 nsinq_sb[:, t, :])
            kst = p1sb.tile([128, HD], BF16, tag="kst", bufs=3,
                            name=f"kst_{t}")
            v_eng.tensor_add(kst[:, 0:RD], tck, tsk)
            v_eng.tensor_copy(out=kst[:, RD:HD], in_=kn[:, RD:HD])
            kt_ps = p1ps.tile([128, 128], BF16, tag="kv_ps", bufs=1,
                              name=f"ktps_{t}")
            nc.tensor.transpose(kt_ps, kst, ident)
            nc.scalar.activation(out=kT_sb[:, t * 128:(t + 1) * 128],
                                 in_=kt_ps, func=AFT.Copy)

        # ---- block j=0 attention, run entirely inside phase 1 ----------
        # scores/exp/den on the spare sc1 bank pair, attn@v into y0 (the 2
        # banks freed by the q_ps/kv_ps shrink), tail + out-projection
        # borrowing the sc1 slot.  All of it fills phase-1 dependency gaps.
        j0_state = {}
        j0_den = [None, None]

        def tail_j0(hp):
            ysb = p1sb.tile([128, 2, BLK], BF16, tag="ysb0", bufs=2,
                            name=f"ysb0_{hp}")
            nc.vector.tensor_copy(out=ysb, in_=j0_state.pop("y0"))
            for hh in range(2):
                h = hp * 2 + hh
                dn_ps = p1ps.tile([1, BLK], F32, tag="sc1_ps", bufs=1,
                                  name=f"dnps0_{h}")
                mm(dn_ps, lhsT=ones_col, rhs=j0_den[hp][:, hh, :],
                   start=True, stop=True)
                rdr = p1sb.tile([1, BLK], BF16, tag="rdr0", bufs=2,
                                name=f"rdr0_{h}")
                nc.vector.reciprocal(rdr, dn_ps)
                rdb_ps = p1ps.tile([128, BLK], F32, tag="sc1_ps", bufs=1,
                                   name=f"rdbps0_{h}")
                mm(rdb_ps, lhsT=ones_row, rhs=rdr, start=True, stop=True)
                nc.vector.tensor_mul(yT_sb[:, h, 0:BLK], ysb[:, hh, :],
                                     rdb_ps)

        def outproj_j0(oc):
            osl = slice(oc * 128, (oc + 1) * 128)
            # alternate between the two spare-bank tags so two
            # out-projection chains overlap
            ops_ = p1ps.tile([128, BLK], F32,
                             tag=("sc1_ps" if oc % 2 else "y0_ps"), bufs=1,
                             name=f"ops0_{oc}")
            for c in range(NH):
                mm(ops_, lhsT=wo_sb[:, c, osl], rhs=yT_sb[:, c, 0:BLK],
                   start=(c == 0), stop=(c == NH - 1))
            oc_sb = p1sb.tile([128, BLK], BF16, tag="oc0_sb", bufs=4,
                              name=f"ocsb0_{oc}")
            if oc % 2 == 0:
                nc.scalar.activation(out=oc_sb, in_=ops_, func=AFT.Copy)
            else:
                nc.vector.tensor_copy(out=oc_sb, in_=ops_)
            nc.sync.dma_start(out=outT[osl, 0:BLK], in_=oc_sb)

        def j0_unit(u):
            hp, t = divmod(u, 4)
            m = t * 128
            tsl = slice(t * 128, (t + 1) * 128)
            sc2 = p1ps.tile([128, 2, BLK], F32, tag="sc1_ps", bufs=1,
                            name=f"sc1_{u}")
            et = p1sb.tile([128, 2, BLK], BF16, tag="et0", bufs=3,
                           name=f"et0_{u}")
            for hh in range(2):
                h = hp * 2 + hh
                mm(sc2[:, hh, m:BLK], lhsT=kT_sb[:, tsl],
                   rhs=qT_sb[:, h, m:BLK], start=True, stop=True)
            nc.scalar.activation(out=et[:, :, m:BLK], in_=sc2[:, :, m:BLK],
                                 func=AFT.Exp, scale=rstdk_sb[:, t:t + 1])
            nc.gpsimd.tensor_mul(
                et[:, :, m:m + 128], et[:, :, m:m + 128],
                tri[:, None, :].broadcast_to([128, 2, 128]))
            if t == 0:
                den = p1sb.tile([128, 2, BLK], BF16, tag="den0", bufs=2,
                                name=f"den0_{hp}")
                j0_den[hp] = den
                nc.vector.tensor_copy(out=den, in_=et)
                j0_state["y0"] = p1ps.tile([128, 2, BLK], F32, tag="y0_ps",
                                           bufs=1, name=f"y0_{hp}")
            else:
                nc.vector.tensor_add(j0_den[hp][:, :, m:BLK],
                                     j0_den[hp][:, :, m:BLK],
                                     et[:, :, m:BLK])
            y0 = j0_state["y0"]
            for hh in range(2):
                mm(y0[:, hh, m:BLK], lhsT=kv_sb[:, t, HD:2 * HD],
                   rhs=et[:, hh, m:BLK], start=(t == 0), stop=(t == 3))
            if t == 3:
                tail_j0(hp)

        j0q = ([(lambda u=u: j0_unit(u)) for u in range(8)] +
               [(lambda oc=oc: outproj_j0(oc)) for oc in range(8)])
        for b in range(NB):
            sl = slice(b * BLK, (b + 1) * BLK)
            xts = []
            for di in range(NDC):
                xt = p1sb.tile([128, BLK], BF16, tag="xt", bufs=16,
                               name=f"xt_{b}_{di}")
                nc.sync.dma_start(out=xt,
                                  in_=xT[di * 128:(di + 1) * 128, sl])
                xts.append(xt)
            if b == 0:
                for di in range(NDC):
                    nc.sync.dma_start(out=wkv_sb[:, di, :],
                                      in_=wkv[:, di, :])
                # wo isn't needed until the first out-projection; issuing
                # it late (split in 4 so transfers parallelize across DMA
                # engines) keeps startup DMA bandwidth for x and wkv.
                for c in range(NH):
                    nc.sync.dma_start(out=wo_sb[:, c, :], in_=wo[:, c, :])
            for half in range(2):
                i0 = b * 4 + half * 2
                qps2 = [p1ps.tile([128, BLK], F32, tag="q_ps", bufs=3,
                                  name=f"qps_{b}_{half}_{x}")
                        for x in range(2)]
                # both x-chunks' kv accumulators packed into one bank
                kvp = p1ps.tile([128, 2, 2 * HD], F32, tag="kv_ps", bufs=1,
                                name=f"kvp_{b}_{half}")
                # kv groups must be contiguous per x (one pending PSUM
                # accumulation group per tile), so kv x=0 runs with q x=0,
                # then kv x=1 with q x=1
                for x in range(2):
                    ii = half * 2 + x
                    for di in range(NDC):
                        st, sp = di == 0, di == NDC - 1
                        xsl = xts[di][:, ii * 128:(ii + 1) * 128]
                        mm(qps2[x], lhsT=xsl, rhs=wq_sb[:, di, :],
                           start=st, stop=sp)
                        mm(kvp[:, x, :], lhsT=xsl, rhs=wkv_sb[:, di, :],
                           start=st, stop=sp)
                # one DVE copy evacuates both kv tiles and frees the bank
                nc.vector.tensor_copy(out=kv_sb[:, i0:i0 + 2, :], in_=kvp)
                for x in range(2):
                    proc_q(i0 + x, qps2[x])
                    proc_k(i0 + x)
                # j=0 attention + out-projection fill phase-1 gaps
                n_u = 0 if b == 0 else (2 if b < 3 else 4)
                for _ in range(n_u):
                    if j0q:
                        j0q.pop(0)()
        while j0q:
            j0q.pop(0)()

    # ---- phase 2+3: attention + per-block output projection ---------------
    # PSUM (8 banks): sc_ps [128,2,512] x2 (4 banks) + y_ps [128,2,512] x2
    # (4 banks); tail dn/rdb and out-projection tiles borrow sc_ps slots.
    with tc.tile_pool(name="p2_psum", bufs=1, space="PSUM") as p2ps, \
         tc.tile_pool(name="p2_sbuf", bufs=1) as p2sb:

        def outproj(j, oc):
            def thunk():
                jsl = slice(j * BLK, (j + 1) * BLK)
                osl = slice(oc * 128, (oc + 1) * 128)
                ops_ = p2ps.tile([128, BLK], F32, tag="aux_ps", bufs=2,
                                 name=f"ops_{j}_{oc}")
                for c in range(NH):
                    mm(ops_, lhsT=wo_sb[:, c, osl], rhs=yT_sb[:, c, jsl],
                       start=(c == 0), stop=(c == NH - 1))
                oc_sb = p2sb.tile([128, BLK], BF16, tag="oc_sb", bufs=6,
                                  name=f"ocsb_{j}_{oc}")
                if oc % 2 == 0:
                    nc.scalar.activation(out=oc_sb, in_=ops_, func=AFT.Copy)
                else:
                    nc.vector.tensor_copy(out=oc_sb, in_=ops_)
                nc.sync.dma_start(out=outT[osl, jsl], in_=oc_sb)
            return thunk

        def tail2(j, hp, yps, den_e, den_o, jsl):
            # softmax tail for this pass's two heads.  One DVE copy frees
            # the y PSUM bank immediately (the next pass's attn@v waits on
            # it); normalization then reads SBUF x rdb-PSUM (one PSUM input).
            ysb = p2sb.tile([128, 2, BLK], BF16, tag="ysb", bufs=2,
                            name=f"ysb_{j}_{hp}")
            nc.vector.tensor_copy(out=ysb, in_=yps)
            for hh in range(2):
                h = hp * 2 + hh
                dn_ps = p2ps.tile([1, BLK], F32, tag="aux_ps", bufs=2,
                                  name=f"dnps_{j}_{h}")
                mm(dn_ps, lhsT=ones_col, rhs=den_e[:, hh, :],
                   start=True, stop=(den_o is None))
                if den_o is not None:
                    mm(dn_ps, lhsT=ones_col, rhs=den_o[:, hh, :],
                       start=False, stop=True)
                rdr = p2sb.tile([1, BLK], BF16, tag="rdr", bufs=4,
                                name=f"rdr_{j}_{h}")
                nc.vector.reciprocal(rdr, dn_ps)
                rdb_ps = p2ps.tile([128, BLK], F32, tag="aux_ps",
                                   bufs=2, name=f"rdbps_{j}_{h}")
                mm(rdb_ps, lhsT=ones_row, rhs=rdr, start=True, stop=True)
                nc.vector.tensor_mul(yT_sb[:, h, jsl], ysb[:, hh, :],
                                     rdb_ps)

        pending = []
        for j in range(1, NB):
            n_t = 4 * j + 4
            jsl = slice(j * BLK, (j + 1) * BLK)
            work, wi = pending, 0
            for hp in range(2):          # head-pair pass (heads 2hp, 2hp+1)
                yps = p2ps.tile([128, 2, BLK], F32, tag="y_ps", bufs=1,
                                name=f"yps_{j}_{hp}")
                den_e = p2sb.tile([128, 2, BLK], BF16, tag="den_e", bufs=2,
                                  name=f"dene_{j}_{hp}")
                den_o = p2sb.tile([128, 2, BLK], BF16, tag="den_o", bufs=2,
                                  name=f"deno_{j}_{hp}")
                for t in range(n_t):
                    diag = t >= 4 * j
                    m = (t - 4 * j) * 128 if diag else 0
                    tsl = slice(t * 128, (t + 1) * 128)
                    sc2 = p2ps.tile([128, 2, BLK], F32, tag="sc_ps",
                                    bufs=2, name=f"sc2_{j}_{hp}_{t}")
                    et2 = p2sb.tile([128, 2, BLK], BF16, tag="expt",
                                    bufs=6, name=f"expt_{j}_{hp}_{t}")
                    for hh in range(2):
                        h = hp * 2 + hh
                        mm(sc2[:, hh, m:BLK], lhsT=kT_sb[:, tsl],
                           rhs=qT_sb[:, h, j * BLK + m:(j + 1) * BLK],
                           start=True, stop=True)
                    nc.scalar.activation(out=et2[:, :, m:BLK],
                                         in_=sc2[:, :, m:BLK], func=AFT.Exp,
                                         scale=rstdk_sb[:, t:t + 1])
                    if diag:
                        nc.gpsimd.tensor_mul(
                            et2[:, :, m:m + 128], et2[:, :, m:m + 128],
                            tri[:, None, :].broadcast_to([128, 2, 128]))
                    if t == 0:
                        nc.vector.tensor_copy(out=den_e, in_=et2)
                    elif j == 0 or t % 2 == 0:
                        nc.vector.tensor_add(den_e[:, :, m:BLK],
                                             den_e[:, :, m:BLK],
                                             et2[:, :, m:BLK])
                    elif t == 1:
                        nc.vector.tensor_copy(out=den_o, in_=et2)
                    else:
                        nc.vector.tensor_add(den_o[:, :, m:BLK],
                                             den_o[:, :, m:BLK],
                                             et2[:, :, m:BLK])
                    for hh in range(2):
                        mm(yps[:, hh, m:BLK], lhsT=kv_sb[:, t, HD:2 * HD],
                           rhs=et2[:, hh, m:BLK],
                           start=(t == 0), stop=(t == n_t - 1))
                    if wi < len(work):
                        work[wi]()
                        wi += 1
                tail2(j, hp, yps, den_e, den_o, jsl)
            while wi < len(work):
                work[wi]()
                wi += 1
            pending = [outproj(j, oc) for oc in range(8)]
        for thunk in pending:
            thunk()

    persist_cm.__exit__(None, None, None)


_NC_CACHE = {}


def _get_nc():
    if "nc" not in _NC_CACHE:
        _NC_CACHE["nc"] = _build_nc()
    return _NC_CACHE["nc"]


def _host_tables():
    pos = np.arange(S, dtype=np.float32)
    inv = (1.0 / (10000.0 ** (np.arange(0, RD, 2, dtype=np.float32) / RD)))
    fr = np.outer(pos, inv).astype(np.float32)          # [S, 32]
    cos, sin = np.cos(fr), np.sin(fr)
    tile128 = lambda a: np.ascontiguousarray(
        a.reshape(NT, 128, RH).transpose(1, 0, 2)).astype(BFNP)
    return tile128(cos), tile128(sin), tile128(-sin)


def core_in_map(x, w_q, w_k, w_v, w_o, q_gain, core):
    """Host-side per-core input prep: shard + transpose + bf16 convert."""
    cosq, sinq, nsinq = _host_tables()
    b, g = divmod(core, 2)
    cols = slice(g * NH * HD, (g + 1) * NH * HD)

    def wtile(wT, chunks, width):
        # [chunks*128, width] -> [128, chunks, width] bf16
        return np.ascontiguousarray(
            wT.reshape(chunks, 128, width).transpose(1, 0, 2)).astype(BFNP)

    xTc = np.ascontiguousarray(x[b].T).astype(BFNP)             # [D, S]
    wq_t = wtile(np.ascontiguousarray(w_q[cols, :].T), NDC, NH * HD)
    wkv_t = wtile(np.ascontiguousarray(np.concatenate(
        [w_k[g * HD:(g + 1) * HD, :].T, w_v[g * HD:(g + 1) * HD, :].T],
        axis=1)), NDC, 2 * HD)
    wo_t = wtile(np.ascontiguousarray(w_o[:, cols].T), NH, D)
    qsc_h = (q_gain[g * NH:(g + 1) * NH] *
             np.float32(HD ** -0.5)).astype(np.float32).reshape(1, NH)
    return dict(xT=xTc, wq=wq_t, wkv=wkv_t, wo=wo_t,
                cosq=cosq, sinq=sinq, nsinq=nsinq, qsc=qsc_h)


def kernel(x, w_q, w_k, w_v, w_o, q_gain):
    x = np.asarray(x, dtype=np.float32)
    w_q = np.asarray(w_q, dtype=np.float32)
    w_k = np.asarray(w_k, dtype=np.float32)
    w_v = np.asarray(w_v, dtype=np.float32)
    w_o = np.asarray(w_o, dtype=np.float32)
    q_gain = np.asarray(q_gain, dtype=np.float32)

    nc = _get_nc()
    in_maps = [core_in_map(x, w_q, w_k, w_v, w_o, q_gain, core)
               for core in range(8)]
    res = bass_utils.run_bass_kernel_spmd(nc, in_maps,
                                          core_ids=list(range(8)))
    out = np.empty((B, S, D), dtype=np.float32)
    for b in range(B):
        p0 = res.results[2 * b]["outT"].astype(np.float32)
        p1 = res.results[2 * b + 1]["outT"].astype(np.float32)
        out[b] = (p0 + p1).T
    return out
